# revision 26
# baseline (speedup 1.0000x reference)
"""GAT (GATConv + BN + ReLU + Linear + BN + ReLU) on 8 Trainium2 NeuronCores.

Strategy (dst-sharded graph parallel, bf16 data path):
  - Nodes sharded by destination across 8 cores (6250 dst nodes each).
  - Phase 1 is sharded: each core computes xh = x_shard @ W (bf16) for its
    own 6250 nodes plus the attention scalars a_s/a_d, then an AllGather
    builds the full 50000-row bf16 xh table (768B rows: 256 xh in o-major
    head-interleaved layout + 4 a_s + pad) in every core's HBM.
  - Phase 2: edges are grouped by dst-block (128 dst nodes); per block the
    source rows are fetched with dma_gather (768B/row), a_d via a 256B
    dst-local gather, messages scaled by exp(leaky(e)) (bf16 2x DVE mode),
    and aggregated via bf16 indicator matmuls accumulating in PSUM, which
    also produce the softmax denominators. Self-loops are applied in the
    block epilogue. BatchNorm statistics are all-reduced across cores.
  - Head-interleaved (o-major) column layout: col o*4+h holds head h,
    out-feat o. Host permutes W, biases, BN params and W_lin rows to match,
    so the final output is in natural order.
"""
import numpy as np
from contextlib import nullcontext

import concourse.bass as bass
import concourse.mybir as mybir
import concourse.tile as tile
from concourse import bacc
from concourse.bass_utils import run_bass_kernel_spmd

F32 = mybir.dt.float32
F32R = mybir.dt.float32r
BF16 = mybir.dt.bfloat16
I16 = mybir.dt.int16
AF = mybir.ActivationFunctionType
OP = mybir.AluOpType

# problem constants
N = 50000
E = 800000
IN_FEATS = 128
OUT_FEATS = 64
HEADS = 4
HID = 256
NEG_SLOPE = 0.2
EPS = 1e-5
NUM_CORES = 8
ND = N // NUM_CORES          # 6250 dst nodes per core
LO = 32768                   # int16 index split
ROW = 384                    # xh row: 256 xh | 4 a_s | 124 pad  (768B bf16)
P = 128


def _wrap16(arr):
    a = np.asarray(arr, dtype=np.int16)
    assert a.size % 16 == 0
    if a.size == 0:
        return np.zeros((128, 1), np.int16)
    w = a.reshape(-1, 16).T.copy()
    return np.tile(w, (8, 1))


def _wrap128(arr):
    a = np.asarray(arr, dtype=np.float32)
    assert a.size % 128 == 0
    if a.size == 0:
        return np.zeros((128, 1), np.float32)
    return a.reshape(-1, 128).T.copy()


def _phi(v, n):
    """Table-row permutation: within each phase-1 store chunk of CB blocks,
    node j*128+p is stored at row p*bn+j (contiguous per-partition stores)."""
    v = np.asarray(v, np.int64)
    CB = 8
    nfull = n // P
    nch = nfull // CB
    full_end = nch * CB * P            # 49152
    part_bn = nfull - nch * CB         # blocks in the partial chunk
    part_end = nfull * P               # 49920
    out = np.empty_like(v)
    m0 = v < full_end
    q = v[m0] % (CB * P)
    out[m0] = (v[m0] // (CB * P)) * (CB * P) + (q % P) * CB + q // P
    m1 = (v >= full_end) & (v < part_end)
    q = v[m1] - full_end
    out[m1] = full_end + (q % P) * part_bn + q // P
    m2 = v >= part_end
    out[m2] = v[m2]
    return out


def host_prep(x, edge_index, W_gat, att_src, att_dst, bias_gat,
              bn1_gamma, bn1_beta, W_lin, b_lin, bn2_gamma, bn2_beta,
              n=N, e=E, num_cores=NUM_CORES):
    """Build per-core padded edge structures + constant tiles."""
    nd = n // num_cores
    nb = (nd + P - 1) // P                     # dst blocks per core
    src = np.asarray(edge_index[0], dtype=np.int64)
    dst = np.asarray(edge_index[1], dtype=np.int64)

    per_core = []
    lo_cnt = np.zeros((num_cores, nb), np.int64)
    hi_cnt = np.zeros((num_cores, nb), np.int64)
    for c in range(num_cores):
        perm = np.concatenate([
            np.arange(c * nd, (c + 1) * nd),
            np.arange(0, c * nd),
            np.arange((c + 1) * nd, n),
        ])
        pinv = np.empty(n, np.int64)
        pinv[perm] = np.arange(n)
        m = (dst >= c * nd) & (dst < (c + 1) * nd)
        es, ed = _phi(pinv[src[m]], n), dst[m] - c * nd
        blk = ed >> 7
        ishi = (es >= LO).astype(np.int64)
        order = np.lexsort((es, ishi, blk))
        es, ed, blk, ishi = es[order], ed[order], blk[order], ishi[order]
        for b in range(nb):
            bm = blk == b
            lo_cnt[c, b] = int(np.sum(bm & (ishi == 0)))
            hi_cnt[c, b] = int(np.sum(bm & (ishi == 1)))
        per_core.append((perm, es, ed, blk, ishi))

    def _pad_to(v):
        return int(-(-v // P) * P)

    m_lo = [_pad_to(int(lo_cnt[:, b].max())) for b in range(nb)]
    m_hi = [_pad_to(int(hi_cnt[:, b].max())) for b in range(nb)]
    g_b = [(m_lo[b] + m_hi[b]) // P for b in range(nb)]

    core_data = []
    for c in range(num_cores):
        perm, es, ed, blk, ishi = per_core[c]
        idx_lo, idx_hi, idx_ad, dstl = [], [], [], []
        for b in range(nb):
            bm_lo = (blk == b) & (ishi == 0)
            bm_hi = (blk == b) & (ishi == 1)
            pl = es[bm_lo]
            ph = es[bm_hi] - LO
            dl = ed[bm_lo] & 127
            dh = ed[bm_hi] & 127
            al = _phi(ed[bm_lo], n)
            ah = _phi(ed[bm_hi], n)
            npl = m_lo[b] - len(pl)
            nph = m_hi[b] - len(ph)
            idx_lo.append(np.concatenate([pl, np.zeros(npl, np.int64)]))
            idx_hi.append(np.concatenate([ph, np.zeros(nph, np.int64)]))
            idx_ad.append(np.concatenate([al, np.zeros(npl, np.int64),
                                          ah, np.zeros(nph, np.int64)]))
            dstl.append(np.concatenate([dl, np.full(npl, 300.0),
                                        dh, np.full(nph, 300.0)]))
        core_data.append(dict(
            x_t=np.ascontiguousarray(
                np.asarray(x, np.float32)[perm].T),
            idx_lo=_wrap16(np.concatenate(idx_lo)),
            idx_hi=_wrap16(np.concatenate(idx_hi)),
            idx_ad=_wrap16(np.concatenate(idx_ad)),
            dstl=_wrap128(np.concatenate(dstl)),
        ))

    # constants (shared by all cores), o-major head-interleaved layout
    import ml_dtypes
    bf = ml_dtypes.bfloat16
    for cd in core_data:
        cd["x_t"] = np.ascontiguousarray(cd["x_t"].astype(bf))

    W_gat = np.asarray(W_gat, np.float32)          # [128, 4, 64]
    att_src = np.asarray(att_src, np.float32)
    att_dst = np.asarray(att_dst, np.float32)
    V_s = np.einsum("iho,ho->ih", W_gat, att_src).astype(np.float32)
    V_d = np.einsum("iho,ho->ih", W_gat, att_dst).astype(np.float32)
    W_om = W_gat.transpose(0, 2, 1).reshape(IN_FEATS, HID)   # col o*4+h
    wvv = np.concatenate([W_om, V_s, V_d], axis=1)           # [128, 264]

    pm_idx = (np.arange(HID).reshape(HEADS, OUT_FEATS).T.reshape(-1))
    # pm_idx[o*4+h] = h*64+o : maps o-major col -> natural col
    bias_om = np.asarray(bias_gat, np.float32)[pm_idx]
    g1_om = np.asarray(bn1_gamma, np.float32)[pm_idx]
    b1_om = np.asarray(bn1_beta, np.float32)[pm_idx]
    Wl_om = np.asarray(W_lin, np.float32)[pm_idx, :]         # rows permuted

    consts = dict(
        wvv=np.ascontiguousarray(wvv).astype(bf),
        iota=np.tile(np.arange(P, dtype=np.float32)[None, :], (P, 1)),
        ident=np.eye(P, dtype=np.float32),
        ident_bf=np.eye(P, dtype=np.float32).astype(bf),
        ones_col=np.ones((P, 1), np.float32),
        ones_row=np.ones((1, P), np.float32),
        g1=g1_om.reshape(2, P).T.copy(),
        b1=b1_om.reshape(2, P).T.copy(),
        g2=np.asarray(bn2_gamma, np.float32)[:, None].copy(),
        b2=np.asarray(bn2_beta, np.float32)[:, None].copy(),
        wlin=np.ascontiguousarray(
            Wl_om.reshape(2, P, OUT_FEATS).transpose(1, 0, 2)
            .reshape(P, 2 * OUT_FEATS)),
    )
    struct = dict(n=n, nd=nd, nb=nb, m_lo=m_lo, m_hi=m_hi, g_b=g_b,
                  num_cores=num_cores)
    return struct, core_data, consts


class StopPhases(Exception):
    pass


def build_kernel(struct, reps=1, skip_cc=False, stop_after=4, probe=None):
    n = struct["n"]
    nd = struct["nd"]
    nb = struct["nb"]
    m_lo = struct["m_lo"]
    m_hi = struct["m_hi"]
    g_b = struct["g_b"]
    num_cores = struct["num_cores"]
    L_lo = sum(m_lo)
    L_hi = sum(m_hi)
    L_ad = L_lo + L_hi
    G = sum(g_b)

    nc = bacc.Bacc("TRN2", debug=False, num_devices=num_cores,
                   dynamic_dma_scratch_size=49152, num_swdge_queues=3)

    # I/O
    x_t = nc.dram_tensor("x_t", [IN_FEATS, n], BF16, kind="ExternalInput")
    idx_lo = nc.dram_tensor("idx_lo", [P, max(L_lo // 16, 1)], I16, kind="ExternalInput")
    idx_hi = nc.dram_tensor("idx_hi", [P, max(L_hi // 16, 1)], I16, kind="ExternalInput")
    idx_ad = nc.dram_tensor("idx_ad", [P, max(L_ad // 16, 1)], I16, kind="ExternalInput")
    dstl_d = nc.dram_tensor("dstl", [P, G], F32, kind="ExternalInput")
    wvv_d = nc.dram_tensor("wvv", [IN_FEATS, HID + 8], BF16, kind="ExternalInput")
    iota_d = nc.dram_tensor("iota", [P, P], F32, kind="ExternalInput")
    ident_d = nc.dram_tensor("ident", [P, P], F32, kind="ExternalInput")
    identb_d = nc.dram_tensor("ident_bf", [P, P], BF16, kind="ExternalInput")
    onesc_d = nc.dram_tensor("ones_col", [P, 1], F32, kind="ExternalInput")
    onesr_d = nc.dram_tensor("ones_row", [1, P], F32, kind="ExternalInput")
    g1_d = nc.dram_tensor("g1", [P, 2], F32, kind="ExternalInput")
    b1_d = nc.dram_tensor("b1", [P, 2], F32, kind="ExternalInput")
    g2_d = nc.dram_tensor("g2", [OUT_FEATS, 1], F32, kind="ExternalInput")
    b2_d = nc.dram_tensor("b2", [OUT_FEATS, 1], F32, kind="ExternalInput")
    wlin_d = nc.dram_tensor("wlin", [P, 2 * OUT_FEATS], F32, kind="ExternalInput")
    y_d = nc.dram_tensor("y", [nd, OUT_FEATS], F32, kind="ExternalOutput")
    debug = struct.get("debug", False)
    if debug:
        dbg_h = nc.dram_tensor("dbg_h", [nd, HID], F32, kind="ExternalOutput")
        dbg_o = nc.dram_tensor("dbg_o", [nd, OUT_FEATS], F32, kind="ExternalOutput")
        dbg_st = nc.dram_tensor("dbg_st", [P, 8], F32, kind="ExternalOutput")

    # internals
    xh_ext = nc.dram_tensor("xh_ext", [n, ROW], BF16)
    asd = nc.dram_tensor("asd", [n, 64], F32)
    bn1_in = nc.dram_tensor("bn1_in", [P, 4], F32)
    bn1_out = nc.dram_tensor("bn1_out", [P, 4], F32)
    bn2_in = nc.dram_tensor("bn2_in", [OUT_FEATS, 2], F32)
    bn2_out = nc.dram_tensor("bn2_out", [OUT_FEATS, 2], F32)

    rg = [list(range(num_cores))]

    with tile.TileContext(nc) as tc:
        with tc.tile_pool(name="const", bufs=1) as cpool, \
             tc.tile_pool(name="resid", bufs=1) as rpool:
            # constants
            wvv_t = cpool.tile([IN_FEATS, HID + 8], BF16)
            nc.sync.dma_start(out=wvv_t[:], in_=wvv_d[:])
            iota_t = cpool.tile([P, P], F32)
            nc.sync.dma_start(out=iota_t[:], in_=iota_d[:])
            ident_t = cpool.tile([P, P], F32)
            nc.sync.dma_start(out=ident_t[:], in_=ident_d[:])
            identb_t = cpool.tile([P, P], BF16)
            nc.sync.dma_start(out=identb_t[:], in_=identb_d[:])
            onesc_t = cpool.tile([P, 1], F32)
            nc.sync.dma_start(out=onesc_t[:], in_=onesc_d[:])
            onesr_t = cpool.tile([1, P], F32)
            nc.sync.dma_start(out=onesr_t[:], in_=onesr_d[:])
            g1_t = cpool.tile([P, 2], F32)
            nc.sync.dma_start(out=g1_t[:], in_=g1_d[:])
            b1_t = cpool.tile([P, 2], F32)
            nc.sync.dma_start(out=b1_t[:], in_=b1_d[:])
            g2_t = cpool.tile([OUT_FEATS, 1], F32)
            nc.sync.dma_start(out=g2_t[:], in_=g2_d[:])
            b2_t = cpool.tile([OUT_FEATS, 1], F32)
            nc.sync.dma_start(out=b2_t[:], in_=b2_d[:])
            wlin_t = cpool.tile([P, 2 * OUT_FEATS], F32)
            nc.sync.dma_start(out=wlin_t[:], in_=wlin_d[:])

            # residents
            h_res = rpool.tile([P, nb * HID], F32)
            o2_res = rpool.tile([P, nb * OUT_FEATS], F32)
            idx_lo_t = rpool.tile([P, max(L_lo // 16, 1)], I16)
            nc.sync.dma_start(out=idx_lo_t[:], in_=idx_lo[:])
            idx_hi_t = rpool.tile([P, max(L_hi // 16, 1)], I16)
            nc.sync.dma_start(out=idx_hi_t[:], in_=idx_hi[:])
            idx_ad_t = rpool.tile([P, max(L_ad // 16, 1)], I16)
            nc.sync.dma_start(out=idx_ad_t[:], in_=idx_ad[:])
            dstl_t = rpool.tile([P, G], F32)
            nc.sync.dma_start(out=dstl_t[:], in_=dstl_d[:])

            loop_cm = tc.For_i(0, reps, 1) if reps > 1 else nullcontext()
            with loop_cm:
                try:
                    # ---- phase 1: xh_ext = x_perm @ W (bf16), replicated ----
                    # DMA-batched: CH blocks per load/store to unload the SP
                    # sequencer (565ns per dma_start issue).
                    nblk1 = (n + P - 1) // P
                    CH = 8
                    nfull = n // P              # full 128-row blocks
                    nch = nfull // CH           # whole chunks of CH blocks
                    with tc.tile_pool(name="p1s", bufs=3) as p1s, \
                         tc.tile_pool(name="p1pt", bufs=2, space="PSUM") as p1pt, \
                         tc.tile_pool(name="p1pm", bufs=2, space="PSUM") as p1pm:
                        for ci in range(nch + 1):
                            b0 = ci * CH
                            bn_ = min(CH, nfull - b0)
                            if bn_ <= 0:
                                break
                            r0 = b0 * P
                            xb = p1s.tile([P, CH * P], BF16, tag="xb")
                            nc.sync.dma_start(
                                out=xb[:, 0:bn_ * P],
                                in_=x_t[:, r0:r0 + bn_ * P])
                            sbx = p1s.tile([P, CH, ROW], BF16, tag="sbx")
                            sba = p1s.tile([P, CH, 64], F32, tag="sba")
                            for j in range(bn_):
                                i = b0 + j
                                pm = p1pm.tile([P, HID + 8], F32, tag="pm")
                                nc.tensor.matmul(out=pm[:],
                                                 lhsT=xb[:, j * P:(j + 1) * P],
                                                 rhs=wvv_t[:],
                                                 start=True, stop=True)
                                if i % 2 == 0:
                                    nc.scalar.copy(sbx[:, j, 0:HID + 4],
                                                   pm[:, 0:HID + 4])
                                    nc.vector.tensor_copy(sba[:, j, 0:8],
                                                          pm[:, HID:HID + 8])
                                else:
                                    nc.vector.tensor_copy(sbx[:, j, 0:HID + 4],
                                                          pm[:, 0:HID + 4])
                                    nc.scalar.copy(sba[:, j, 0:8],
                                                   pm[:, HID:HID + 8])
                            if probe != "nostore":
                                # phi layout: DRAM row r0+p*bn_+j <- node r0+j*128+p;
                                # full rows so each partition's bn_ rows coalesce
                                nc.sync.dma_start(
                                    out=xh_ext[r0:r0 + bn_ * P, :].rearrange(
                                        "(p j) f -> p j f", j=bn_),
                                    in_=sbx[:, 0:bn_, :])
                                nc.sync.dma_start(
                                    out=asd[r0:r0 + bn_ * P, :].rearrange(
                                        "(p j) f -> p j f", j=bn_),
                                    in_=sba[:, 0:bn_, :])
                            else:
                                nc.sync.dma_start(out=xh_ext[r0:r0 + P, 0:HID + 4],
                                                  in_=sbx[:, 0, 0:HID + 4])
                        if n % P:
                            r0 = nfull * P
                            rn = n - r0
                            xb = p1s.tile([P, CH * P], BF16, tag="xb")
                            nc.sync.dma_start(out=xb[:, 0:rn],
                                              in_=x_t[:, r0:r0 + rn])
                            pm = p1pm.tile([P, HID + 8], F32, tag="pm")
                            nc.tensor.matmul(out=pm[:rn], lhsT=xb[:, 0:rn], rhs=wvv_t[:],
                                             start=True, stop=True)
                            sbx = p1s.tile([P, CH, ROW], BF16, tag="sbx")
                            sba = p1s.tile([P, CH, 64], F32, tag="sba")
                            nc.scalar.copy(sbx[:rn, 0, 0:HID + 4], pm[:rn, 0:HID + 4])
                            nc.vector.tensor_copy(sba[:rn, 0, 0:8],
                                                  pm[:rn, HID:HID + 8])
                            nc.sync.dma_start(out=xh_ext[r0:r0 + rn, 0:HID + 4],
                                              in_=sbx[:rn, 0, 0:HID + 4])
                            nc.sync.dma_start(out=asd[r0:r0 + rn, 0:8],
                                              in_=sba[:rn, 0, 0:8])

                    # ---- phase 2: edge aggregation ----
                    if stop_after < 2:
                        raise StopPhases
                    with tc.tile_pool(name="p2g", bufs=2) as p2g, \
                         tc.tile_pool(name="p2a", bufs=2) as p2a, \
                         tc.tile_pool(name="p2i", bufs=3) as p2i, \
                         tc.tile_pool(name="p2s", bufs=3) as p2s, \
                         tc.tile_pool(name="p2p", bufs=2, space="PSUM") as p2p, \
                         tc.tile_pool(name="p2st", bufs=1, space="PSUM") as p2st:
                        ps_stats = [p2st.tile([P, 1], F32, tag=f"st{j}", name=f"st{j}")
                                    for j in range(4)]
                        off_lo = 0
                        off_hi = 0
                        off_ad = 0
                        gof = 0
                        ISUB = 6  # indicator groups per DVE op
                        gmax = max(g_b)
                        for b in range(nb):
                            nd_b = min(P, nd - b * P)
                            glo = m_lo[b] // P
                            ghi = m_hi[b] // P
                            gb = g_b[b]
                            gath = p2g.tile([P, gmax, ROW], BF16, tag="gath")
                            if probe in ("xh512", "none"):
                                gp = p2g.tile([P, gmax, 256], BF16, tag="gprobe")
                                if m_lo[b] > 0:
                                    nc.gpsimd.dma_gather(
                                        out_ap=gp[:, 0:glo, :], in_ap=xh_ext[0:min(LO, n), 0:256],
                                        idxs_ap=idx_lo_t[:, off_lo:off_lo + m_lo[b] // 16],
                                        num_idxs=m_lo[b], num_idxs_reg=m_lo[b],
                                        elem_size=256, elem_step=ROW, single_packet=False)
                                if m_hi[b] > 0:
                                    nc.gpsimd.dma_gather(
                                        out_ap=gp[:, glo:gb, :], in_ap=xh_ext[LO:n, 0:256],
                                        idxs_ap=idx_hi_t[:, off_hi:off_hi + m_hi[b] // 16],
                                        num_idxs=m_hi[b], num_idxs_reg=m_hi[b],
                                        elem_size=256, elem_step=ROW, single_packet=False)
                            else:
                                if m_lo[b] > 0:
                                    nc.gpsimd.dma_gather(
                                        out_ap=gath[:, 0:glo, :], in_ap=xh_ext[0:min(LO, n), :],
                                        idxs_ap=idx_lo_t[:, off_lo:off_lo + m_lo[b] // 16],
                                        num_idxs=m_lo[b], num_idxs_reg=m_lo[b],
                                        elem_size=ROW, single_packet=False,
                                        queue_num=0)
                                if m_hi[b] > 0:
                                    nc.gpsimd.dma_gather(
                                        out_ap=gath[:, glo:gb, :], in_ap=xh_ext[LO:n, :],
                                        idxs_ap=idx_hi_t[:, off_hi:off_hi + m_hi[b] // 16],
                                        num_idxs=m_hi[b], num_idxs_reg=m_hi[b],
                                        elem_size=ROW, single_packet=False,
                                        queue_num=1)
                            ad_g = p2a.tile([P, gmax, 64], F32, tag="adg")
                            if probe not in ("noad", "none"):
                                nc.gpsimd.dma_gather(
                                    out_ap=ad_g[:, 0:gb, :], in_ap=asd[0:8192, :],
                                    idxs_ap=idx_ad_t[:, off_ad:off_ad + (m_lo[b] + m_hi[b]) // 16],
                                    num_idxs=m_lo[b] + m_hi[b], num_idxs_reg=m_lo[b] + m_hi[b],
                                    elem_size=64, single_packet=False, queue_num=2)
                            # self-loop data (phi layout: one load per 8 blocks)
                            if b % 8 == 0:
                                c0 = (b // 8) * 8 * P
                                xh_blk8 = p2s.tile([P, 8, ROW], BF16, tag="xhb8")
                                nc.sync.dma_start(
                                    out=xh_blk8[:],
                                    in_=xh_ext[c0:c0 + 8 * P, :].rearrange(
                                        "(p j) f -> p j f", j=8))
                                asd_blk8 = p2s.tile([P, 8, 64], F32, tag="asdb8")
                                nc.sync.dma_start(
                                    out=asd_blk8[:],
                                    in_=asd[c0:c0 + 8 * P, :].rearrange(
                                        "(p j) f -> p j f", j=8))
                            xh_blk = xh_blk8[:, b % 8, :]
                            asd_blk = asd_blk8[:, b % 8, 0:8]

                            # ee = exp(leaky(a_s[src] + a_d[dst]))  [128, gb, 4]
                            ee = p2s.tile([P, gmax, 4], BF16, tag="ee")
                            nc.vector.tensor_tensor(ee[:, 0:gb, :],
                                                    gath[:, 0:gb, HID:HID + 4],
                                                    ad_g[:, 0:gb, 4:8], OP.add)
                            nc.vector.scalar_tensor_tensor(
                                ee[:, 0:gb, :], ee[:, 0:gb, :], NEG_SLOPE, ee[:, 0:gb, :],
                                OP.mult, OP.max)
                            nc.scalar.activation(ee[:, 0:gb, :], ee[:, 0:gb, :], AF.Exp)
                            # scale message in place (o-major bf16: 2x DVE)
                            nc.vector.tensor_tensor(
                                gath[:, 0:gb, 0:HID].rearrange("p g (o h) -> p g o h", h=HEADS),
                                gath[:, 0:gb, 0:HID].rearrange("p g (o h) -> p g o h", h=HEADS),
                                ee[:, 0:gb, None, :].to_broadcast([P, gb, OUT_FEATS, HEADS]),
                                OP.mult)
                            nc.scalar.copy(gath[:, 0:gb, HID:HID + 4], ee[:, 0:gb, :])

                            # indicators
                            ind = []
                            for j0 in range(0, gb, ISUB):
                                j1 = min(j0 + ISUB, gb)
                                it = p2i.tile([P, ISUB, P], BF16, tag="ind")
                                nc.vector.tensor_tensor(
                                    it[:, 0:j1 - j0, :],
                                    iota_t[:, None, :].to_broadcast([P, j1 - j0, P]),
                                    dstl_t[:, gof + j0:gof + j1, None].to_broadcast(
                                        [P, j1 - j0, P]),
                                    OP.is_equal)
                                ind.append((j0, it))
                            psb = p2p.tile([P, HID + 4], F32, tag="psb")
                            for g in range(gb):
                                it = ind[g // ISUB][1]
                                nc.tensor.matmul(
                                    out=psb[:nd_b], lhsT=it[:, g % ISUB, 0:nd_b],
                                    rhs=gath[:, g, 0:HID + 4],
                                    start=(g == 0), stop=(g == gb - 1))

                            # epilogue: self loops, normalize, bias, h, stats
                            ee_s = p2s.tile([P, 4], F32, tag="ees")
                            nc.vector.tensor_tensor(ee_s[:nd_b], asd_blk[:nd_b, 0:4],
                                                    asd_blk[:nd_b, 4:8], OP.add)
                            nc.vector.scalar_tensor_tensor(
                                ee_s[:nd_b], ee_s[:nd_b], NEG_SLOPE, ee_s[:nd_b],
                                OP.mult, OP.max)
                            nc.scalar.activation(ee_s[:nd_b], ee_s[:nd_b], AF.Exp)
                            den = p2s.tile([P, 4], F32, tag="den")
                            nc.vector.tensor_tensor(den[:nd_b], psb[:nd_b, HID:HID + 4],
                                                    ee_s[:nd_b], OP.add)
                            rec = p2s.tile([P, 4], F32, tag="rec")
                            nc.vector.reciprocal(rec[:nd_b], den[:nd_b])
                            t1 = p2s.tile([P, HID], F32, tag="t1")
                            nc.vector.tensor_tensor(
                                t1[:nd_b].rearrange("p (o h) -> p o h", h=HEADS),
                                xh_blk[:nd_b, 0:HID].rearrange("p (o h) -> p o h", h=HEADS),
                                ee_s[:nd_b, None, :].to_broadcast([nd_b, OUT_FEATS, HEADS]),
                                OP.mult)
                            nc.vector.tensor_tensor(t1[:nd_b], t1[:nd_b],
                                                    psb[:nd_b, 0:HID], OP.add)
                            hslot = h_res[:, b * HID:(b + 1) * HID]
                            nc.vector.tensor_tensor(
                                hslot[:nd_b].rearrange("p (o h) -> p o h", h=HEADS),
                                t1[:nd_b].rearrange("p (o h) -> p o h", h=HEADS),
                                rec[:nd_b, None, :].to_broadcast([nd_b, OUT_FEATS, HEADS]),
                                OP.mult)
                            if debug:
                                nc.sync.dma_start(out=dbg_h[b * P:b * P + nd_b, :],
                                                  in_=hslot[:nd_b])
                            sq = p2s.tile([P, HID], F32, tag="sq")
                            nc.vector.tensor_tensor(sq[:nd_b], hslot[:nd_b], hslot[:nd_b],
                                                    OP.mult)
                            for k in range(2):
                                nc.tensor.matmul(out=ps_stats[k][:],
                                                 lhsT=hslot[:nd_b, k * P:(k + 1) * P],
                                                 rhs=onesc_t[:nd_b],
                                                 start=(b == 0), stop=(b == nb - 1))
                                nc.tensor.matmul(out=ps_stats[2 + k][:],
                                                 lhsT=sq[:nd_b, k * P:(k + 1) * P],
                                                 rhs=onesc_t[:nd_b],
                                                 start=(b == 0), stop=(b == nb - 1))
                            off_lo += m_lo[b] // 16
                            off_hi += m_hi[b] // 16
                            off_ad += (m_lo[b] + m_hi[b]) // 16
                            gof += gb

                        # BN1 stats allreduce + s,t
                        st_sb = p2s.tile([P, 4], F32, tag="stsb")
                        for j in range(4):
                            nc.vector.tensor_copy(st_sb[:, j:j + 1], ps_stats[j][:])
                        nc.sync.dma_start(out=bn1_in[:], in_=st_sb[:])
                        if not skip_cc:
                            nc.gpsimd.collective_compute(
                                "AllReduce", OP.add, replica_groups=rg,
                                ins=[bn1_in[:]], outs=[bn1_out[:]])
                        else:
                            nc.sync.dma_start(out=bn1_out[:], in_=st_sb[:])
                        st_g = p2s.tile([P, 4], F32, tag="stg")
                        nc.sync.dma_start(out=st_g[:], in_=bn1_out[:])

                    if stop_after < 3:
                        raise StopPhases
                    with tc.tile_pool(name="p3s", bufs=3) as p3s, \
                         tc.tile_pool(name="bc", bufs=1) as bc, \
                         tc.tile_pool(name="p3pt", bufs=2, space="PSUM") as p3pt, \
                         tc.tile_pool(name="p3po", bufs=2, space="PSUM") as p3po, \
                         tc.tile_pool(name="p3st", bufs=1, space="PSUM") as p3st, \
                         tc.tile_pool(name="p3bc", bufs=1, space="PSUM") as p3bc:
                        mean = p3s.tile([P, 2], F32, tag="mean")
                        nc.scalar.mul(mean[:], st_g[:, 0:2], 1.0 / n)
                        esq = p3s.tile([P, 2], F32, tag="esq")
                        nc.scalar.mul(esq[:], st_g[:, 2:4], 1.0 / n)
                        var = p3s.tile([P, 2], F32, tag="var")
                        nc.vector.tensor_tensor(var[:], mean[:], mean[:], OP.mult)
                        nc.vector.tensor_tensor(var[:], esq[:], var[:], OP.subtract)
                        nc.vector.tensor_scalar_add(var[:], var[:], EPS)
                        sdv = p3s.tile([P, 2], F32, tag="sdv")
                        nc.scalar.activation(sdv[:], var[:], AF.Sqrt)
                        inv = p3s.tile([P, 2], F32, tag="inv")
                        nc.vector.reciprocal(inv[:], sdv[:])
                        s1 = p3s.tile([P, 2], F32, tag="s1")
                        nc.vector.tensor_tensor(s1[:], inv[:], g1_t[:], OP.mult)
                        tsh = p3s.tile([P, 2], F32, tag="tsh")
                        nc.vector.tensor_tensor(tsh[:], mean[:], s1[:], OP.mult)
                        nc.vector.tensor_tensor(tsh[:], b1_t[:], tsh[:], OP.subtract)

                        # broadcast s1/tsh to node-major [P, 256]
                        s_bc = bc.tile([P, HID], F32)
                        t_bc = bc.tile([P, HID], F32)
                        for (vec, dstt) in ((s1, s_bc), (tsh, t_bc)):
                            for k in range(2):
                                row = p3s.tile([1, P], F32, tag="row")
                                nc.sync.dma_start(out=row[:], in_=vec[:, k:k + 1])
                                pbc = p3bc.tile([P, P], F32, tag="pbc")
                                nc.tensor.matmul(out=pbc[:], lhsT=onesr_t[:], rhs=row[:],
                                                 start=True, stop=True)
                                nc.scalar.copy(dstt[:, k * P:(k + 1) * P], pbc[:])

                        if debug:
                            nc.sync.dma_start(out=dbg_st[:, 0:4], in_=st_g[:])
                            nc.sync.dma_start(out=dbg_st[:, 4:6], in_=s1[:])
                            nc.sync.dma_start(out=dbg_st[:, 6:8], in_=tsh[:])
                        # ---- phase 3: BN1 + relu + linear + BN2 stats ----
                        ps_st2 = [p3st.tile([OUT_FEATS, 1], F32, tag=f"st2{j}",
                                            name=f"st2{j}")[:] for j in range(2)]
                        for b in range(nb):
                            nd_b = min(P, nd - b * P)
                            hslot = h_res[:, b * HID:(b + 1) * HID]
                            hb = p3s.tile([P, HID], F32, tag="hb")
                            nc.vector.tensor_tensor(hb[:nd_b], hslot[:nd_b], s_bc[:nd_b],
                                                    OP.mult)
                            nc.vector.tensor_tensor(hb[:nd_b], hb[:nd_b], t_bc[:nd_b],
                                                    OP.add)
                            nc.vector.tensor_scalar(hb[:nd_b], hb[:nd_b], 0.0, None,
                                                    OP.max)
                            po = p3po.tile([P, OUT_FEATS], F32, tag="po")
                            for k in range(2):
                                ptr = p3pt.tile([P, P], F32, tag="tr")
                                nc.tensor.transpose(out=ptr[:, :nd_b],
                                                    in_=hb[:nd_b, k * P:(k + 1) * P],
                                                    identity=ident_t[:nd_b, :nd_b])
                                hbt = p3s.tile([P, P], F32, tag="hbt")
                                if k == 0:
                                    nc.scalar.copy(hbt[:, :nd_b], ptr[:, :nd_b])
                                else:
                                    nc.vector.tensor_copy(hbt[:, :nd_b], ptr[:, :nd_b])
                                nc.tensor.matmul(out=po[:nd_b], lhsT=hbt[:, :nd_b],
                                                 rhs=wlin_t[:, k * OUT_FEATS:(k + 1) * OUT_FEATS],
                                                 start=(k == 0), stop=(k == 1))
                            oslot = o2_res[:, b * OUT_FEATS:(b + 1) * OUT_FEATS]
                            if b % 2 == 0:
                                nc.vector.tensor_copy(oslot[:nd_b], po[:nd_b])
                            else:
                                nc.scalar.copy(oslot[:nd_b], po[:nd_b])
                            if debug:
                                nc.sync.dma_start(out=dbg_o[b * P:b * P + nd_b, :],
                                                  in_=oslot[:nd_b])
                            sq2 = p3s.tile([P, OUT_FEATS], F32, tag="sq2")
                            nc.vector.tensor_tensor(sq2[:nd_b], oslot[:nd_b],
                                                    oslot[:nd_b], OP.mult)
                            nc.tensor.matmul(out=ps_st2[0], lhsT=oslot[:nd_b],
                                             rhs=onesc_t[:nd_b],
                                             start=(b == 0), stop=(b == nb - 1))
                            nc.tensor.matmul(out=ps_st2[1], lhsT=sq2[:nd_b],
                                             rhs=onesc_t[:nd_b],
                                             start=(b == 0), stop=(b == nb - 1))

                        st2_sb = p3s.tile([OUT_FEATS, 2], F32, tag="st2sb")
                        for j in range(2):
                            nc.vector.tensor_copy(st2_sb[:, j:j + 1], ps_st2[j])
                        nc.sync.dma_start(out=bn2_in[:], in_=st2_sb[:])
                        if not skip_cc:
                            nc.gpsimd.collective_compute(
                                "AllReduce", OP.add, replica_groups=rg,
                                ins=[bn2_in[:]], outs=[bn2_out[:]])
                        else:
                            nc.sync.dma_start(out=bn2_out[:], in_=st2_sb[:])
                        st2_g = p3s.tile([OUT_FEATS, 2], F32, tag="st2g")
                        nc.sync.dma_start(out=st2_g[:], in_=bn2_out[:])

                        mean2 = p3s.tile([OUT_FEATS, 1], F32, tag="mean2")
                        nc.scalar.mul(mean2[:], st2_g[:, 0:1], 1.0 / n)
                        esq2 = p3s.tile([OUT_FEATS, 1], F32, tag="esq2")
                        nc.scalar.mul(esq2[:], st2_g[:, 1:2], 1.0 / n)
                        var2 = p3s.tile([OUT_FEATS, 1], F32, tag="var2")
                        nc.vector.tensor_tensor(var2[:], mean2[:], mean2[:], OP.mult)
                        nc.vector.tensor_tensor(var2[:], esq2[:], var2[:], OP.subtract)
                        nc.vector.tensor_scalar_add(var2[:], var2[:], EPS)
                        sdv2 = p3s.tile([OUT_FEATS, 1], F32, tag="sdv2")
                        nc.scalar.activation(sdv2[:], var2[:], AF.Sqrt)
                        inv2 = p3s.tile([OUT_FEATS, 1], F32, tag="inv2")
                        nc.vector.reciprocal(inv2[:], sdv2[:])
                        s2 = p3s.tile([OUT_FEATS, 1], F32, tag="s2")
                        nc.vector.tensor_tensor(s2[:], inv2[:], g2_t[:], OP.mult)
                        t2 = p3s.tile([OUT_FEATS, 1], F32, tag="t2")
                        nc.vector.tensor_tensor(t2[:], mean2[:], s2[:], OP.mult)
                        nc.vector.tensor_tensor(t2[:], b2_t[:], t2[:], OP.subtract)

                        s2_bc = bc.tile([P, OUT_FEATS], F32)
                        t2_bc = bc.tile([P, OUT_FEATS], F32)
                        for (vec, dstt) in ((s2, s2_bc), (t2, t2_bc)):
                            row = p3s.tile([1, OUT_FEATS], F32, tag="row2")
                            nc.sync.dma_start(out=row[:], in_=vec[:])
                            pbc = p3bc.tile([P, P], F32, tag="pbc")
                            nc.tensor.matmul(out=pbc[:, 0:OUT_FEATS], lhsT=onesr_t[:],
                                             rhs=row[:], start=True, stop=True)
                            nc.scalar.copy(dstt[:], pbc[:, 0:OUT_FEATS])

                        # ---- phase 4: BN2 apply + relu + store ----
                        for b in range(nb):
                            nd_b = min(P, nd - b * P)
                            oslot = o2_res[:, b * OUT_FEATS:(b + 1) * OUT_FEATS]
                            ob = p3s.tile([P, OUT_FEATS], F32, tag="ob")
                            nc.vector.tensor_tensor(ob[:nd_b], oslot[:nd_b], s2_bc[:nd_b],
                                                    OP.mult)
                            nc.vector.tensor_tensor(ob[:nd_b], ob[:nd_b], t2_bc[:nd_b],
                                                    OP.add)
                            nc.vector.tensor_scalar(ob[:nd_b], ob[:nd_b], 0.0, None,
                                                    OP.max)
                            nc.sync.dma_start(out=y_d[b * P:b * P + nd_b, :],
                                              in_=ob[:nd_b])

                except StopPhases:
                    pass
    nc.compile()
    return nc


def _legalize_waits(nc, max_waits=1):
    """This walrus build encodes at most one sync-wait per instruction; move
    extra waits onto preceding NoOps on the same engine."""
    nsplit = 0
    for bb in nc.main_func.blocks:
        new = []
        for ins in bb.instructions:
            si = ins.sync_info
            if si is not None and len(si.on_wait) > max_waits:
                waits = list(si.on_wait)
                for j, w in enumerate(waits[max_waits:]):
                    nop = mybir.InstNoOp(
                        name=f"{ins.name}_wsplit{j}", ins=[], outs=[],
                        engine=ins.engine,
                        sync_info=mybir.SyncInfo(on_wait=[w], on_update=[]),
                    )
                    new.append(nop)
                    nsplit += 1
                si.on_wait = waits[:max_waits]
            new.append(ins)
        bb.instructions[:] = new
    return nsplit


def kernel(**inputs):
    x = np.asarray(inputs["x"], np.float32)
    edge_index = np.asarray(inputs["edge_index"])
    struct, core_data, consts = host_prep(
        x, edge_index, inputs["W_gat"], inputs["att_src"], inputs["att_dst"],
        inputs["bias_gat"], inputs["bn1_gamma"], inputs["bn1_beta"],
        inputs["W_lin"], inputs["b_lin"], inputs["bn2_gamma"], inputs["bn2_beta"])
    nc = build_kernel(struct)
    _legalize_waits(nc)
    in_maps = []
    for c in range(struct["num_cores"]):
        m = dict(consts)
        m.update(core_data[c])
        in_maps.append(m)
    res = run_bass_kernel_spmd(nc, in_maps, list(range(struct["num_cores"])))
    out = np.concatenate([res.results[c]["y"] for c in range(struct["num_cores"])],
                         axis=0)
    return out.astype(np.float32)


# revision 30
# speedup vs baseline: 1.1181x; 1.1181x over previous
"""GAT (GATConv + BN + ReLU + Linear + BN + ReLU) on 8 Trainium2 NeuronCores.

Strategy (dst-sharded graph parallel, bf16 data path):
  - Nodes sharded by destination across 8 cores (6250 dst nodes each).
  - Phase 1 is sharded: each core computes xh = x_shard @ W (bf16) for its
    own 6250 nodes plus the attention scalars a_s/a_d, then an AllGather
    builds the full 50000-row bf16 xh table (768B rows: 256 xh in o-major
    head-interleaved layout + 4 a_s + pad) in every core's HBM.
  - Phase 2: edges are grouped by dst-block (128 dst nodes); per block the
    source rows are fetched with dma_gather (768B/row), a_d via a 256B
    dst-local gather, messages scaled by exp(leaky(e)) (bf16 2x DVE mode),
    and aggregated via bf16 indicator matmuls accumulating in PSUM, which
    also produce the softmax denominators. Self-loops are applied in the
    block epilogue. BatchNorm statistics are all-reduced across cores.
  - Head-interleaved (o-major) column layout: col o*4+h holds head h,
    out-feat o. Host permutes W, biases, BN params and W_lin rows to match,
    so the final output is in natural order.
"""
import numpy as np
from contextlib import nullcontext

import concourse.bass as bass
import concourse.mybir as mybir
import concourse.tile as tile
from concourse import bacc
from concourse.bass_utils import run_bass_kernel_spmd

F32 = mybir.dt.float32
F32R = mybir.dt.float32r
BF16 = mybir.dt.bfloat16
I16 = mybir.dt.int16
AF = mybir.ActivationFunctionType
OP = mybir.AluOpType

# problem constants
N = 50000
E = 800000
IN_FEATS = 128
OUT_FEATS = 64
HEADS = 4
HID = 256
NEG_SLOPE = 0.2
EPS = 1e-5
NUM_CORES = 8
ND = N // NUM_CORES          # 6250 dst nodes per core
LO = 32768                   # int16 index split
ROW = 384                    # xh row: 256 xh | 4 a_s | 124 pad  (768B bf16)
P = 128


def _wrap16(arr):
    a = np.asarray(arr, dtype=np.int16)
    assert a.size % 16 == 0
    if a.size == 0:
        return np.zeros((128, 1), np.int16)
    w = a.reshape(-1, 16).T.copy()
    return np.tile(w, (8, 1))


def _wrap128(arr):
    a = np.asarray(arr, dtype=np.float32)
    assert a.size % 128 == 0
    if a.size == 0:
        return np.zeros((128, 1), np.float32)
    return a.reshape(-1, 128).T.copy()


def _phi(v, n):
    """Table-row permutation: within each phase-1 store chunk of CB blocks,
    node j*128+p is stored at row p*bn+j (contiguous per-partition stores)."""
    v = np.asarray(v, np.int64)
    CB = 8
    nfull = n // P
    nch = nfull // CB
    full_end = nch * CB * P            # 49152
    part_bn = nfull - nch * CB         # blocks in the partial chunk
    part_end = nfull * P               # 49920
    out = np.empty_like(v)
    m0 = v < full_end
    q = v[m0] % (CB * P)
    out[m0] = (v[m0] // (CB * P)) * (CB * P) + (q % P) * CB + q // P
    m1 = (v >= full_end) & (v < part_end)
    q = v[m1] - full_end
    out[m1] = full_end + (q % P) * part_bn + q // P
    m2 = v >= part_end
    out[m2] = v[m2]
    return out


def host_prep(x, edge_index, W_gat, att_src, att_dst, bias_gat,
              bn1_gamma, bn1_beta, W_lin, b_lin, bn2_gamma, bn2_beta,
              n=N, e=E, num_cores=NUM_CORES):
    """Build per-core padded edge structures + constant tiles."""
    nd = n // num_cores
    nb = (nd + P - 1) // P                     # dst blocks per core
    src = np.asarray(edge_index[0], dtype=np.int64)
    dst = np.asarray(edge_index[1], dtype=np.int64)

    per_core = []
    lo_cnt = np.zeros((num_cores, nb), np.int64)
    hi_cnt = np.zeros((num_cores, nb), np.int64)
    for c in range(num_cores):
        perm = np.concatenate([
            np.arange(c * nd, (c + 1) * nd),
            np.arange(0, c * nd),
            np.arange((c + 1) * nd, n),
        ])
        pinv = np.empty(n, np.int64)
        pinv[perm] = np.arange(n)
        m = (dst >= c * nd) & (dst < (c + 1) * nd)
        es, ed = _phi(pinv[src[m]], n), dst[m] - c * nd
        blk = ed >> 7
        ishi = (es >= LO).astype(np.int64)
        order = np.lexsort((es, ishi, blk))
        es, ed, blk, ishi = es[order], ed[order], blk[order], ishi[order]
        for b in range(nb):
            bm = blk == b
            lo_cnt[c, b] = int(np.sum(bm & (ishi == 0)))
            hi_cnt[c, b] = int(np.sum(bm & (ishi == 1)))
        per_core.append((perm, es, ed, blk, ishi))

    def _pad_to(v):
        return int(-(-v // P) * P)

    m_lo = [_pad_to(int(lo_cnt[:, b].max())) for b in range(nb)]
    m_hi = [_pad_to(int(hi_cnt[:, b].max())) for b in range(nb)]
    g_b = [(m_lo[b] + m_hi[b]) // P for b in range(nb)]

    core_data = []
    for c in range(num_cores):
        perm, es, ed, blk, ishi = per_core[c]
        idx_lo, idx_hi, idx_ad, dstl = [], [], [], []
        for b in range(nb):
            bm_lo = (blk == b) & (ishi == 0)
            bm_hi = (blk == b) & (ishi == 1)
            pl = es[bm_lo]
            ph = es[bm_hi] - LO
            dl = ed[bm_lo] & 127
            dh = ed[bm_hi] & 127
            al = _phi(ed[bm_lo], n)
            ah = _phi(ed[bm_hi], n)
            npl = m_lo[b] - len(pl)
            nph = m_hi[b] - len(ph)
            idx_lo.append(np.concatenate([pl, np.zeros(npl, np.int64)]))
            idx_hi.append(np.concatenate([ph, np.zeros(nph, np.int64)]))
            idx_ad.append(np.concatenate([al, np.zeros(npl, np.int64),
                                          ah, np.zeros(nph, np.int64)]))
            dstl.append(np.concatenate([dl, np.full(npl, 300.0),
                                        dh, np.full(nph, 300.0)]))
        core_data.append(dict(
            x_t=np.ascontiguousarray(
                np.asarray(x, np.float32)[perm].T),
            idx_lo=_wrap16(np.concatenate(idx_lo)),
            idx_hi=_wrap16(np.concatenate(idx_hi)),
            dstl=_wrap128(np.concatenate(dstl)),
        ))

    # constants (shared by all cores), o-major head-interleaved layout
    import ml_dtypes
    bf = ml_dtypes.bfloat16
    for cd in core_data:
        cd["x_t"] = np.ascontiguousarray(cd["x_t"].astype(bf))

    W_gat = np.asarray(W_gat, np.float32)          # [128, 4, 64]
    att_src = np.asarray(att_src, np.float32)
    att_dst = np.asarray(att_dst, np.float32)
    V_s = np.einsum("iho,ho->ih", W_gat, att_src).astype(np.float32)
    V_d = np.einsum("iho,ho->ih", W_gat, att_dst).astype(np.float32)
    W_om = W_gat.transpose(0, 2, 1).reshape(IN_FEATS, HID)   # col o*4+h
    wvv = np.concatenate([W_om, V_s, V_d], axis=1)           # [128, 264]

    pm_idx = (np.arange(HID).reshape(HEADS, OUT_FEATS).T.reshape(-1))
    # pm_idx[o*4+h] = h*64+o : maps o-major col -> natural col
    bias_om = np.asarray(bias_gat, np.float32)[pm_idx]
    g1_om = np.asarray(bn1_gamma, np.float32)[pm_idx]
    b1_om = np.asarray(bn1_beta, np.float32)[pm_idx]
    Wl_om = np.asarray(W_lin, np.float32)[pm_idx, :]         # rows permuted

    consts = dict(
        wvv=np.ascontiguousarray(wvv).astype(bf),
        iota=np.tile(np.arange(P, dtype=np.float32)[None, :], (P, 1)),
        ident=np.eye(P, dtype=np.float32),
        ident_bf=np.eye(P, dtype=np.float32).astype(bf),
        ones_col=np.ones((P, 1), np.float32),
        zeros32=np.zeros((P, 32), np.float32),
        ones_row=np.ones((1, P), np.float32),
        g1=g1_om.reshape(2, P).T.copy(),
        b1=b1_om.reshape(2, P).T.copy(),
        g2=np.asarray(bn2_gamma, np.float32)[:, None].copy(),
        b2=np.asarray(bn2_beta, np.float32)[:, None].copy(),
        wlin=np.ascontiguousarray(
            Wl_om.reshape(2, P, OUT_FEATS).transpose(1, 0, 2)
            .reshape(P, 2 * OUT_FEATS)),
    )
    struct = dict(n=n, nd=nd, nb=nb, m_lo=m_lo, m_hi=m_hi, g_b=g_b,
                  num_cores=num_cores)
    return struct, core_data, consts


class StopPhases(Exception):
    pass


def build_kernel(struct, reps=1, skip_cc=False, stop_after=4, probe=None):
    n = struct["n"]
    nd = struct["nd"]
    nb = struct["nb"]
    m_lo = struct["m_lo"]
    m_hi = struct["m_hi"]
    g_b = struct["g_b"]
    num_cores = struct["num_cores"]
    L_lo = sum(m_lo)
    L_hi = sum(m_hi)
    L_ad = L_lo + L_hi
    G = sum(g_b)

    nc = bacc.Bacc("TRN2", debug=False, num_devices=num_cores,
                   dynamic_dma_scratch_size=49152, num_swdge_queues=3)

    # I/O
    x_t = nc.dram_tensor("x_t", [IN_FEATS, n], BF16, kind="ExternalInput")
    idx_lo = nc.dram_tensor("idx_lo", [P, max(L_lo // 16, 1)], I16, kind="ExternalInput")
    idx_hi = nc.dram_tensor("idx_hi", [P, max(L_hi // 16, 1)], I16, kind="ExternalInput")
    dstl_d = nc.dram_tensor("dstl", [P, G], F32, kind="ExternalInput")
    wvv_d = nc.dram_tensor("wvv", [IN_FEATS, HID + 8], BF16, kind="ExternalInput")
    iota_d = nc.dram_tensor("iota", [P, P], F32, kind="ExternalInput")
    ident_d = nc.dram_tensor("ident", [P, P], F32, kind="ExternalInput")
    identb_d = nc.dram_tensor("ident_bf", [P, P], BF16, kind="ExternalInput")
    onesc_d = nc.dram_tensor("ones_col", [P, 1], F32, kind="ExternalInput")
    zeros_d = nc.dram_tensor("zeros32", [P, 32], F32, kind="ExternalInput")
    onesr_d = nc.dram_tensor("ones_row", [1, P], F32, kind="ExternalInput")
    g1_d = nc.dram_tensor("g1", [P, 2], F32, kind="ExternalInput")
    b1_d = nc.dram_tensor("b1", [P, 2], F32, kind="ExternalInput")
    g2_d = nc.dram_tensor("g2", [OUT_FEATS, 1], F32, kind="ExternalInput")
    b2_d = nc.dram_tensor("b2", [OUT_FEATS, 1], F32, kind="ExternalInput")
    wlin_d = nc.dram_tensor("wlin", [P, 2 * OUT_FEATS], F32, kind="ExternalInput")
    y_d = nc.dram_tensor("y", [nd, OUT_FEATS], F32, kind="ExternalOutput")
    debug = struct.get("debug", False)
    if debug:
        dbg_h = nc.dram_tensor("dbg_h", [nd, HID], F32, kind="ExternalOutput")
        dbg_o = nc.dram_tensor("dbg_o", [nd, OUT_FEATS], F32, kind="ExternalOutput")
        dbg_st = nc.dram_tensor("dbg_st", [P, 8], F32, kind="ExternalOutput")

    # internals
    xh_ext = nc.dram_tensor("xh_ext", [n, ROW], BF16)
    asd = nc.dram_tensor("asd", [n, 64], F32)
    bn1_in = nc.dram_tensor("bn1_in", [P, 4], F32)
    bn1_out = nc.dram_tensor("bn1_out", [P, 4], F32)
    bn2_in = nc.dram_tensor("bn2_in", [OUT_FEATS, 2], F32)
    bn2_out = nc.dram_tensor("bn2_out", [OUT_FEATS, 2], F32)

    rg = [list(range(num_cores))]

    with tile.TileContext(nc) as tc:
        with tc.tile_pool(name="const", bufs=1) as cpool, \
             tc.tile_pool(name="resid", bufs=1) as rpool:
            # constants
            wvv_t = cpool.tile([IN_FEATS, HID + 8], BF16)
            nc.sync.dma_start(out=wvv_t[:], in_=wvv_d[:])
            iota_t = cpool.tile([P, P], F32)
            nc.sync.dma_start(out=iota_t[:], in_=iota_d[:])
            ident_t = cpool.tile([P, P], F32)
            nc.sync.dma_start(out=ident_t[:], in_=ident_d[:])
            identb_t = cpool.tile([P, P], BF16)
            nc.sync.dma_start(out=identb_t[:], in_=identb_d[:])
            onesc_t = cpool.tile([P, 1], F32)
            nc.sync.dma_start(out=onesc_t[:], in_=onesc_d[:])
            zeros_t = cpool.tile([P, 32], F32)
            nc.sync.dma_start(out=zeros_t[:], in_=zeros_d[:])
            onesr_t = cpool.tile([1, P], F32)
            nc.sync.dma_start(out=onesr_t[:], in_=onesr_d[:])
            g1_t = cpool.tile([P, 2], F32)
            nc.sync.dma_start(out=g1_t[:], in_=g1_d[:])
            b1_t = cpool.tile([P, 2], F32)
            nc.sync.dma_start(out=b1_t[:], in_=b1_d[:])
            g2_t = cpool.tile([OUT_FEATS, 1], F32)
            nc.sync.dma_start(out=g2_t[:], in_=g2_d[:])
            b2_t = cpool.tile([OUT_FEATS, 1], F32)
            nc.sync.dma_start(out=b2_t[:], in_=b2_d[:])
            wlin_t = cpool.tile([P, 2 * OUT_FEATS], F32)
            nc.sync.dma_start(out=wlin_t[:], in_=wlin_d[:])

            # residents
            h_res = rpool.tile([P, nb * HID], F32)
            o2_res = rpool.tile([P, nb * OUT_FEATS], F32)
            idx_lo_t = rpool.tile([P, max(L_lo // 16, 1)], I16)
            nc.sync.dma_start(out=idx_lo_t[:], in_=idx_lo[:])
            idx_hi_t = rpool.tile([P, max(L_hi // 16, 1)], I16)
            nc.sync.dma_start(out=idx_hi_t[:], in_=idx_hi[:])
            dstl_t = rpool.tile([P, G], F32)
            nc.sync.dma_start(out=dstl_t[:], in_=dstl_d[:])

            loop_cm = tc.For_i(0, reps, 1) if reps > 1 else nullcontext()
            with loop_cm:
                try:
                    # ---- phase 1: xh_ext = x_perm @ W (bf16), replicated ----
                    # DMA-batched: CH blocks per load/store to unload the SP
                    # sequencer (565ns per dma_start issue).
                    nblk1 = (n + P - 1) // P
                    CH = 8
                    nfull = n // P              # full 128-row blocks
                    nch = nfull // CH           # whole chunks of CH blocks
                    with tc.tile_pool(name="p1s", bufs=3) as p1s, \
                         tc.tile_pool(name="p1pt", bufs=2, space="PSUM") as p1pt, \
                         tc.tile_pool(name="p1pm", bufs=2, space="PSUM") as p1pm:
                        for ci in range(nch + 1):
                            b0 = ci * CH
                            bn_ = min(CH, nfull - b0)
                            if bn_ <= 0:
                                break
                            r0 = b0 * P
                            xb = p1s.tile([P, CH * P], BF16, tag="xb")
                            nc.sync.dma_start(
                                out=xb[:, 0:bn_ * P],
                                in_=x_t[:, r0:r0 + bn_ * P])
                            sbx = p1s.tile([P, CH, ROW], BF16, tag="sbx")
                            sba = p1s.tile([P, CH, 64], F32, tag="sba")
                            for j in range(bn_):
                                i = b0 + j
                                pm = p1pm.tile([P, HID + 8], F32, tag="pm")
                                nc.tensor.matmul(out=pm[:],
                                                 lhsT=xb[:, j * P:(j + 1) * P],
                                                 rhs=wvv_t[:],
                                                 start=True, stop=True)
                                if i % 2 == 0:
                                    nc.scalar.copy(sbx[:, j, 0:HID + 4],
                                                   pm[:, 0:HID + 4])
                                    nc.vector.tensor_copy(sba[:, j, 0:8],
                                                          pm[:, HID:HID + 8])
                                else:
                                    nc.vector.tensor_copy(sbx[:, j, 0:HID + 4],
                                                          pm[:, 0:HID + 4])
                                    nc.scalar.copy(sba[:, j, 0:8],
                                                   pm[:, HID:HID + 8])
                            if probe != "nostore":
                                # phi layout: DRAM row r0+p*bn_+j <- node r0+j*128+p;
                                # full rows so each partition's bn_ rows coalesce
                                nc.sync.dma_start(
                                    out=xh_ext[r0:r0 + bn_ * P, :].rearrange(
                                        "(p j) f -> p j f", j=bn_),
                                    in_=sbx[:, 0:bn_, :])
                                nc.sync.dma_start(
                                    out=asd[r0:r0 + bn_ * P, :].rearrange(
                                        "(p j) f -> p j f", j=bn_),
                                    in_=sba[:, 0:bn_, :])
                            else:
                                nc.sync.dma_start(out=xh_ext[r0:r0 + P, 0:HID + 4],
                                                  in_=sbx[:, 0, 0:HID + 4])
                        if n % P:
                            r0 = nfull * P
                            rn = n - r0
                            xb = p1s.tile([P, CH * P], BF16, tag="xb")
                            nc.sync.dma_start(out=xb[:, 0:rn],
                                              in_=x_t[:, r0:r0 + rn])
                            pm = p1pm.tile([P, HID + 8], F32, tag="pm")
                            nc.tensor.matmul(out=pm[:rn], lhsT=xb[:, 0:rn], rhs=wvv_t[:],
                                             start=True, stop=True)
                            sbx = p1s.tile([P, CH, ROW], BF16, tag="sbx")
                            sba = p1s.tile([P, CH, 64], F32, tag="sba")
                            nc.scalar.copy(sbx[:rn, 0, 0:HID + 4], pm[:rn, 0:HID + 4])
                            nc.vector.tensor_copy(sba[:rn, 0, 0:8],
                                                  pm[:rn, HID:HID + 8])
                            nc.sync.dma_start(out=xh_ext[r0:r0 + rn, 0:HID + 4],
                                              in_=sbx[:rn, 0, 0:HID + 4])
                            nc.sync.dma_start(out=asd[r0:r0 + rn, 0:8],
                                              in_=sba[:rn, 0, 0:8])

                    # ---- phase 2: edge aggregation ----
                    if stop_after < 2:
                        raise StopPhases
                    with tc.tile_pool(name="p2g", bufs=2) as p2g, \
                         tc.tile_pool(name="p2i", bufs=4) as p2i, \
                         tc.tile_pool(name="p2s", bufs=3) as p2s, \
                         tc.tile_pool(name="p2p", bufs=2, space="PSUM") as p2p, \
                         tc.tile_pool(name="p2t", bufs=2, space="PSUM") as p2t, \
                         tc.tile_pool(name="p2ad", bufs=2, space="PSUM") as p2ad, \
                         tc.tile_pool(name="p2st", bufs=1, space="PSUM") as p2st:
                        stats_ps = p2st.tile([P, 4], F32, tag="st", name="st")
                        nc.vector.tensor_copy(stats_ps[:], zeros_t[:, 0:4])
                        off_lo = 0
                        off_hi = 0
                        off_ad = 0
                        gof = 0
                        ISUB = 7  # indicator groups per DVE op
                        gmax = max(g_b)
                        for b in range(nb):
                            nd_b = min(P, nd - b * P)
                            glo = m_lo[b] // P
                            ghi = m_hi[b] // P
                            gb = g_b[b]
                            gath = p2g.tile([P, gmax, ROW], BF16, tag="gath")
                            if probe in ("xh512", "none"):
                                gp = p2g.tile([P, gmax, 256], BF16, tag="gprobe")
                                if m_lo[b] > 0:
                                    nc.gpsimd.dma_gather(
                                        out_ap=gp[:, 0:glo, :], in_ap=xh_ext[0:min(LO, n), 0:256],
                                        idxs_ap=idx_lo_t[:, off_lo:off_lo + m_lo[b] // 16],
                                        num_idxs=m_lo[b], num_idxs_reg=m_lo[b],
                                        elem_size=256, elem_step=ROW, single_packet=False)
                                if m_hi[b] > 0:
                                    nc.gpsimd.dma_gather(
                                        out_ap=gp[:, glo:gb, :], in_ap=xh_ext[LO:n, 0:256],
                                        idxs_ap=idx_hi_t[:, off_hi:off_hi + m_hi[b] // 16],
                                        num_idxs=m_hi[b], num_idxs_reg=m_hi[b],
                                        elem_size=256, elem_step=ROW, single_packet=False)
                            else:
                                if m_lo[b] > 0:
                                    nc.gpsimd.dma_gather(
                                        out_ap=gath[:, 0:glo, :], in_ap=xh_ext[0:min(LO, n), :],
                                        idxs_ap=idx_lo_t[:, off_lo:off_lo + m_lo[b] // 16],
                                        num_idxs=m_lo[b], num_idxs_reg=m_lo[b],
                                        elem_size=ROW, single_packet=False,
                                        queue_num=0)
                                if m_hi[b] > 0:
                                    nc.gpsimd.dma_gather(
                                        out_ap=gath[:, glo:gb, :], in_ap=xh_ext[LO:n, :],
                                        idxs_ap=idx_hi_t[:, off_hi:off_hi + m_hi[b] // 16],
                                        num_idxs=m_hi[b], num_idxs_reg=m_hi[b],
                                        elem_size=ROW, single_packet=False,
                                        queue_num=1)
                            # self-loop data (phi layout: one load per 8 blocks)
                            if b % 8 == 0:
                                c0 = (b // 8) * 8 * P
                                xh_blk8 = p2s.tile([P, 8, ROW], BF16, tag="xhb8")
                                nc.sync.dma_start(
                                    out=xh_blk8[:],
                                    in_=xh_ext[c0:c0 + 8 * P, :].rearrange(
                                        "(p j) f -> p j f", j=8))
                                asd_blk8 = p2s.tile([P, 8, 64], F32, tag="asdb8")
                                nc.sync.dma_start(
                                    out=asd_blk8[:],
                                    in_=asd[c0:c0 + 8 * P, :].rearrange(
                                        "(p j) f -> p j f", j=8))
                            xh_blk = xh_blk8[:, b % 8, :]
                            asd_blk = asd_blk8[:, b % 8, 0:8]

                            if probe in ("p2a", "p2a_noad"):
                                off_lo += m_lo[b] // 16
                                off_hi += m_hi[b] // 16
                                gof += gb
                                continue
                            # indicators (ISUB groups per build op) + per-group
                            # a_d via transposed-indicator matmul into adps
                            ee = p2s.tile([P, gmax, 4], BF16, tag="ee")
                            ind = []
                            for j0 in range(0, gb, ISUB):
                                j1 = min(j0 + ISUB, gb)
                                it = p2i.tile([P, ISUB, P], BF16, tag="ind")
                                nc.vector.tensor_tensor(
                                    it[:, 0:j1 - j0, :],
                                    iota_t[:, None, :].to_broadcast([P, j1 - j0, P]),
                                    dstl_t[:, gof + j0:gof + j1, None].to_broadcast(
                                        [P, j1 - j0, P]),
                                    OP.is_equal)
                                ind.append((j0, it))
                                adps = p2ad.tile([P, 4 * ISUB], F32, tag="adps")
                                nc.vector.tensor_copy(adps[:, 0:4 * (j1 - j0)],
                                                      zeros_t[:, 0:4 * (j1 - j0)])
                                for g in range(j0, j1):
                                    ptT = p2t.tile([P, P], BF16, tag="ptT")
                                    nc.tensor.transpose(out=ptT[:], in_=it[:, g - j0, :],
                                                        identity=identb_t[:])
                                    indT = p2s.tile([P, P], F32, tag="indT")
                                    nc.scalar.copy(indT[:], ptT[:])
                                    nc.tensor.matmul(
                                        out=adps[:, 4 * (g - j0):4 * (g - j0) + 4],
                                        lhsT=indT[:], rhs=asd_blk[:, 4:8],
                                        start=False, stop=True)
                                # ee = exp(leaky(a_s[src] + a_d[dst])) for this batch
                                nc.vector.tensor_tensor(
                                    ee[:, j0:j1, :],
                                    gath[:, j0:j1, HID:HID + 4],
                                    adps[:, 0:4 * (j1 - j0)].rearrange(
                                        "p (g q) -> p g q", q=4),
                                    OP.add)
                            nc.vector.scalar_tensor_tensor(
                                ee[:, 0:gb, :], ee[:, 0:gb, :], NEG_SLOPE, ee[:, 0:gb, :],
                                OP.mult, OP.max)
                            nc.scalar.activation(ee[:, 0:gb, :], ee[:, 0:gb, :], AF.Exp)
                            if probe == "p2b":
                                off_lo += m_lo[b] // 16
                                off_hi += m_hi[b] // 16
                                gof += gb
                                continue
                            # scale message in place (o-major bf16: 2x DVE)
                            nc.vector.tensor_tensor(
                                gath[:, 0:gb, 0:HID].rearrange("p g (o h) -> p g o h", h=HEADS),
                                gath[:, 0:gb, 0:HID].rearrange("p g (o h) -> p g o h", h=HEADS),
                                ee[:, 0:gb, None, :].to_broadcast([P, gb, OUT_FEATS, HEADS]),
                                OP.mult)
                            nc.scalar.copy(gath[:, 0:gb, HID:HID + 4], ee[:, 0:gb, :])
                            if probe == "p2c":
                                off_lo += m_lo[b] // 16
                                off_hi += m_hi[b] // 16
                                gof += gb
                                continue
                            psb = p2p.tile([P, HID + 4], F32, tag="psb")
                            for g in range(gb):
                                it = ind[g // ISUB][1]
                                nc.tensor.matmul(
                                    out=psb[:nd_b], lhsT=it[:, g % ISUB, 0:nd_b],
                                    rhs=gath[:, g, 0:HID + 4],
                                    start=(g == 0), stop=(g == gb - 1))

                            # epilogue: self loops, normalize, bias, h, stats
                            ee_s = p2s.tile([P, 4], F32, tag="ees")
                            nc.vector.tensor_tensor(ee_s[:nd_b], asd_blk[:nd_b, 0:4],
                                                    asd_blk[:nd_b, 4:8], OP.add)
                            nc.vector.scalar_tensor_tensor(
                                ee_s[:nd_b], ee_s[:nd_b], NEG_SLOPE, ee_s[:nd_b],
                                OP.mult, OP.max)
                            nc.scalar.activation(ee_s[:nd_b], ee_s[:nd_b], AF.Exp)
                            den = p2s.tile([P, 4], F32, tag="den")
                            nc.vector.tensor_tensor(den[:nd_b], psb[:nd_b, HID:HID + 4],
                                                    ee_s[:nd_b], OP.add)
                            rec = p2s.tile([P, 4], F32, tag="rec")
                            nc.vector.reciprocal(rec[:nd_b], den[:nd_b])
                            t1 = p2s.tile([P, HID], F32, tag="t1")
                            nc.vector.tensor_tensor(
                                t1[:nd_b].rearrange("p (o h) -> p o h", h=HEADS),
                                xh_blk[:nd_b, 0:HID].rearrange("p (o h) -> p o h", h=HEADS),
                                ee_s[:nd_b, None, :].to_broadcast([nd_b, OUT_FEATS, HEADS]),
                                OP.mult)
                            nc.vector.tensor_tensor(t1[:nd_b], t1[:nd_b],
                                                    psb[:nd_b, 0:HID], OP.add)
                            hslot = h_res[:, b * HID:(b + 1) * HID]
                            nc.vector.tensor_tensor(
                                hslot[:nd_b].rearrange("p (o h) -> p o h", h=HEADS),
                                t1[:nd_b].rearrange("p (o h) -> p o h", h=HEADS),
                                rec[:nd_b, None, :].to_broadcast([nd_b, OUT_FEATS, HEADS]),
                                OP.mult)
                            if debug:
                                nc.sync.dma_start(out=dbg_h[b * P:b * P + nd_b, :],
                                                  in_=hslot[:nd_b])
                            sq = p2s.tile([P, HID], F32, tag="sq")
                            nc.vector.tensor_tensor(sq[:nd_b], hslot[:nd_b], hslot[:nd_b],
                                                    OP.mult)
                            for k in range(2):
                                nc.tensor.matmul(out=stats_ps[:, k:k + 1],
                                                 lhsT=hslot[:nd_b, k * P:(k + 1) * P],
                                                 rhs=onesc_t[:nd_b],
                                                 start=False, stop=True)
                                nc.tensor.matmul(out=stats_ps[:, 2 + k:3 + k],
                                                 lhsT=sq[:nd_b, k * P:(k + 1) * P],
                                                 rhs=onesc_t[:nd_b],
                                                 start=False, stop=True)
                            off_lo += m_lo[b] // 16
                            off_hi += m_hi[b] // 16
                            gof += gb

                        if probe in ("p2a", "p2a_noad", "p2b", "p2c"):
                            raise StopPhases
                        # BN1 stats allreduce + s,t
                        st_sb = p2s.tile([P, 4], F32, tag="stsb")
                        nc.vector.tensor_copy(st_sb[:], stats_ps[:])
                        nc.sync.dma_start(out=bn1_in[:], in_=st_sb[:])
                        if not skip_cc:
                            nc.gpsimd.collective_compute(
                                "AllReduce", OP.add, replica_groups=rg,
                                ins=[bn1_in[:]], outs=[bn1_out[:]])
                        else:
                            nc.sync.dma_start(out=bn1_out[:], in_=st_sb[:])
                        st_g = p2s.tile([P, 4], F32, tag="stg")
                        nc.sync.dma_start(out=st_g[:], in_=bn1_out[:])

                    if stop_after < 3:
                        raise StopPhases
                    with tc.tile_pool(name="p3s", bufs=3) as p3s, \
                         tc.tile_pool(name="bc", bufs=1) as bc, \
                         tc.tile_pool(name="p3pt", bufs=2, space="PSUM") as p3pt, \
                         tc.tile_pool(name="p3po", bufs=2, space="PSUM") as p3po, \
                         tc.tile_pool(name="p3st", bufs=1, space="PSUM") as p3st, \
                         tc.tile_pool(name="p3bc", bufs=1, space="PSUM") as p3bc:
                        mean = p3s.tile([P, 2], F32, tag="mean")
                        nc.scalar.mul(mean[:], st_g[:, 0:2], 1.0 / n)
                        esq = p3s.tile([P, 2], F32, tag="esq")
                        nc.scalar.mul(esq[:], st_g[:, 2:4], 1.0 / n)
                        var = p3s.tile([P, 2], F32, tag="var")
                        nc.vector.tensor_tensor(var[:], mean[:], mean[:], OP.mult)
                        nc.vector.tensor_tensor(var[:], esq[:], var[:], OP.subtract)
                        nc.vector.tensor_scalar_add(var[:], var[:], EPS)
                        sdv = p3s.tile([P, 2], F32, tag="sdv")
                        nc.scalar.activation(sdv[:], var[:], AF.Sqrt)
                        inv = p3s.tile([P, 2], F32, tag="inv")
                        nc.vector.reciprocal(inv[:], sdv[:])
                        s1 = p3s.tile([P, 2], F32, tag="s1")
                        nc.vector.tensor_tensor(s1[:], inv[:], g1_t[:], OP.mult)
                        tsh = p3s.tile([P, 2], F32, tag="tsh")
                        nc.vector.tensor_tensor(tsh[:], mean[:], s1[:], OP.mult)
                        nc.vector.tensor_tensor(tsh[:], b1_t[:], tsh[:], OP.subtract)

                        # broadcast s1/tsh to node-major [P, 256]
                        s_bc = bc.tile([P, HID], F32)
                        t_bc = bc.tile([P, HID], F32)
                        for (vec, dstt) in ((s1, s_bc), (tsh, t_bc)):
                            for k in range(2):
                                row = p3s.tile([1, P], F32, tag="row")
                                nc.sync.dma_start(out=row[:], in_=vec[:, k:k + 1])
                                pbc = p3bc.tile([P, P], F32, tag="pbc")
                                nc.tensor.matmul(out=pbc[:], lhsT=onesr_t[:], rhs=row[:],
                                                 start=True, stop=True)
                                nc.scalar.copy(dstt[:, k * P:(k + 1) * P], pbc[:])

                        if debug:
                            nc.sync.dma_start(out=dbg_st[:, 0:4], in_=st_g[:])
                            nc.sync.dma_start(out=dbg_st[:, 4:6], in_=s1[:])
                            nc.sync.dma_start(out=dbg_st[:, 6:8], in_=tsh[:])
                        # ---- phase 3: BN1 + relu + linear + BN2 stats ----
                        ps_st2 = [p3st.tile([OUT_FEATS, 1], F32, tag=f"st2{j}",
                                            name=f"st2{j}")[:] for j in range(2)]
                        for b in range(nb):
                            nd_b = min(P, nd - b * P)
                            hslot = h_res[:, b * HID:(b + 1) * HID]
                            hb = p3s.tile([P, HID], F32, tag="hb")
                            nc.vector.tensor_tensor(hb[:nd_b], hslot[:nd_b], s_bc[:nd_b],
                                                    OP.mult)
                            nc.vector.tensor_tensor(hb[:nd_b], hb[:nd_b], t_bc[:nd_b],
                                                    OP.add)
                            nc.vector.tensor_scalar(hb[:nd_b], hb[:nd_b], 0.0, None,
                                                    OP.max)
                            po = p3po.tile([P, OUT_FEATS], F32, tag="po")
                            for k in range(2):
                                ptr = p3pt.tile([P, P], F32, tag="tr")
                                nc.tensor.transpose(out=ptr[:, :nd_b],
                                                    in_=hb[:nd_b, k * P:(k + 1) * P],
                                                    identity=ident_t[:nd_b, :nd_b])
                                hbt = p3s.tile([P, P], F32, tag="hbt")
                                if k == 0:
                                    nc.scalar.copy(hbt[:, :nd_b], ptr[:, :nd_b])
                                else:
                                    nc.vector.tensor_copy(hbt[:, :nd_b], ptr[:, :nd_b])
                                nc.tensor.matmul(out=po[:nd_b], lhsT=hbt[:, :nd_b],
                                                 rhs=wlin_t[:, k * OUT_FEATS:(k + 1) * OUT_FEATS],
                                                 start=(k == 0), stop=(k == 1))
                            oslot = o2_res[:, b * OUT_FEATS:(b + 1) * OUT_FEATS]
                            if b % 2 == 0:
                                nc.vector.tensor_copy(oslot[:nd_b], po[:nd_b])
                            else:
                                nc.scalar.copy(oslot[:nd_b], po[:nd_b])
                            if debug:
                                nc.sync.dma_start(out=dbg_o[b * P:b * P + nd_b, :],
                                                  in_=oslot[:nd_b])
                            sq2 = p3s.tile([P, OUT_FEATS], F32, tag="sq2")
                            nc.vector.tensor_tensor(sq2[:nd_b], oslot[:nd_b],
                                                    oslot[:nd_b], OP.mult)
                            nc.tensor.matmul(out=ps_st2[0], lhsT=oslot[:nd_b],
                                             rhs=onesc_t[:nd_b],
                                             start=(b == 0), stop=(b == nb - 1))
                            nc.tensor.matmul(out=ps_st2[1], lhsT=sq2[:nd_b],
                                             rhs=onesc_t[:nd_b],
                                             start=(b == 0), stop=(b == nb - 1))

                        st2_sb = p3s.tile([OUT_FEATS, 2], F32, tag="st2sb")
                        for j in range(2):
                            nc.vector.tensor_copy(st2_sb[:, j:j + 1], ps_st2[j])
                        nc.sync.dma_start(out=bn2_in[:], in_=st2_sb[:])
                        if not skip_cc:
                            nc.gpsimd.collective_compute(
                                "AllReduce", OP.add, replica_groups=rg,
                                ins=[bn2_in[:]], outs=[bn2_out[:]])
                        else:
                            nc.sync.dma_start(out=bn2_out[:], in_=st2_sb[:])
                        st2_g = p3s.tile([OUT_FEATS, 2], F32, tag="st2g")
                        nc.sync.dma_start(out=st2_g[:], in_=bn2_out[:])

                        mean2 = p3s.tile([OUT_FEATS, 1], F32, tag="mean2")
                        nc.scalar.mul(mean2[:], st2_g[:, 0:1], 1.0 / n)
                        esq2 = p3s.tile([OUT_FEATS, 1], F32, tag="esq2")
                        nc.scalar.mul(esq2[:], st2_g[:, 1:2], 1.0 / n)
                        var2 = p3s.tile([OUT_FEATS, 1], F32, tag="var2")
                        nc.vector.tensor_tensor(var2[:], mean2[:], mean2[:], OP.mult)
                        nc.vector.tensor_tensor(var2[:], esq2[:], var2[:], OP.subtract)
                        nc.vector.tensor_scalar_add(var2[:], var2[:], EPS)
                        sdv2 = p3s.tile([OUT_FEATS, 1], F32, tag="sdv2")
                        nc.scalar.activation(sdv2[:], var2[:], AF.Sqrt)
                        inv2 = p3s.tile([OUT_FEATS, 1], F32, tag="inv2")
                        nc.vector.reciprocal(inv2[:], sdv2[:])
                        s2 = p3s.tile([OUT_FEATS, 1], F32, tag="s2")
                        nc.vector.tensor_tensor(s2[:], inv2[:], g2_t[:], OP.mult)
                        t2 = p3s.tile([OUT_FEATS, 1], F32, tag="t2")
                        nc.vector.tensor_tensor(t2[:], mean2[:], s2[:], OP.mult)
                        nc.vector.tensor_tensor(t2[:], b2_t[:], t2[:], OP.subtract)

                        s2_bc = bc.tile([P, OUT_FEATS], F32)
                        t2_bc = bc.tile([P, OUT_FEATS], F32)
                        for (vec, dstt) in ((s2, s2_bc), (t2, t2_bc)):
                            row = p3s.tile([1, OUT_FEATS], F32, tag="row2")
                            nc.sync.dma_start(out=row[:], in_=vec[:])
                            pbc = p3bc.tile([P, P], F32, tag="pbc")
                            nc.tensor.matmul(out=pbc[:, 0:OUT_FEATS], lhsT=onesr_t[:],
                                             rhs=row[:], start=True, stop=True)
                            nc.scalar.copy(dstt[:], pbc[:, 0:OUT_FEATS])

                        # ---- phase 4: BN2 apply + relu + store ----
                        for b in range(nb):
                            nd_b = min(P, nd - b * P)
                            oslot = o2_res[:, b * OUT_FEATS:(b + 1) * OUT_FEATS]
                            ob = p3s.tile([P, OUT_FEATS], F32, tag="ob")
                            nc.vector.tensor_tensor(ob[:nd_b], oslot[:nd_b], s2_bc[:nd_b],
                                                    OP.mult)
                            nc.vector.tensor_tensor(ob[:nd_b], ob[:nd_b], t2_bc[:nd_b],
                                                    OP.add)
                            nc.vector.tensor_scalar(ob[:nd_b], ob[:nd_b], 0.0, None,
                                                    OP.max)
                            nc.sync.dma_start(out=y_d[b * P:b * P + nd_b, :],
                                              in_=ob[:nd_b])

                except StopPhases:
                    pass
    nc.compile()
    return nc


def _legalize_waits(nc, max_waits=1):
    """This walrus build encodes at most one sync-wait per instruction; move
    extra waits onto preceding NoOps on the same engine."""
    nsplit = 0
    for bb in nc.main_func.blocks:
        new = []
        for ins in bb.instructions:
            si = ins.sync_info
            if si is not None and len(si.on_wait) > max_waits:
                waits = list(si.on_wait)
                for j, w in enumerate(waits[max_waits:]):
                    nop = mybir.InstNoOp(
                        name=f"{ins.name}_wsplit{j}", ins=[], outs=[],
                        engine=ins.engine,
                        sync_info=mybir.SyncInfo(on_wait=[w], on_update=[]),
                    )
                    new.append(nop)
                    nsplit += 1
                si.on_wait = waits[:max_waits]
            new.append(ins)
        bb.instructions[:] = new
    return nsplit


def kernel(**inputs):
    x = np.asarray(inputs["x"], np.float32)
    edge_index = np.asarray(inputs["edge_index"])
    struct, core_data, consts = host_prep(
        x, edge_index, inputs["W_gat"], inputs["att_src"], inputs["att_dst"],
        inputs["bias_gat"], inputs["bn1_gamma"], inputs["bn1_beta"],
        inputs["W_lin"], inputs["b_lin"], inputs["bn2_gamma"], inputs["bn2_beta"])
    nc = build_kernel(struct)
    _legalize_waits(nc)
    in_maps = []
    for c in range(struct["num_cores"]):
        m = dict(consts)
        m.update(core_data[c])
        in_maps.append(m)
    res = run_bass_kernel_spmd(nc, in_maps, list(range(struct["num_cores"])))
    out = np.concatenate([res.results[c]["y"] for c in range(struct["num_cores"])],
                         axis=0)
    return out.astype(np.float32)


# revision 31
# speedup vs baseline: 1.1808x; 1.0561x over previous
"""GAT (GATConv + BN + ReLU + Linear + BN + ReLU) on 8 Trainium2 NeuronCores.

Strategy (dst-sharded graph parallel, bf16 data path):
  - Nodes sharded by destination across 8 cores (6250 dst nodes each).
  - Phase 1 is sharded: each core computes xh = x_shard @ W (bf16) for its
    own 6250 nodes plus the attention scalars a_s/a_d, then an AllGather
    builds the full 50000-row bf16 xh table (768B rows: 256 xh in o-major
    head-interleaved layout + 4 a_s + pad) in every core's HBM.
  - Phase 2: edges are grouped by dst-block (128 dst nodes); per block the
    source rows are fetched with dma_gather (768B/row), a_d via a 256B
    dst-local gather, messages scaled by exp(leaky(e)) (bf16 2x DVE mode),
    and aggregated via bf16 indicator matmuls accumulating in PSUM, which
    also produce the softmax denominators. Self-loops are applied in the
    block epilogue. BatchNorm statistics are all-reduced across cores.
  - Head-interleaved (o-major) column layout: col o*4+h holds head h,
    out-feat o. Host permutes W, biases, BN params and W_lin rows to match,
    so the final output is in natural order.
"""
import numpy as np
from contextlib import nullcontext

import concourse.bass as bass
import concourse.mybir as mybir
import concourse.tile as tile
from concourse import bacc
from concourse.bass_utils import run_bass_kernel_spmd

F32 = mybir.dt.float32
F32R = mybir.dt.float32r
BF16 = mybir.dt.bfloat16
I16 = mybir.dt.int16
AF = mybir.ActivationFunctionType
OP = mybir.AluOpType

# problem constants
N = 50000
E = 800000
IN_FEATS = 128
OUT_FEATS = 64
HEADS = 4
HID = 256
NEG_SLOPE = 0.2
EPS = 1e-5
NUM_CORES = 8
ND = N // NUM_CORES          # 6250 dst nodes per core
LO = 32768                   # int16 index split
ROW = 384                    # xh row: 256 xh | 4 a_s | 124 pad  (768B bf16)
P = 128


def _wrap16(arr):
    a = np.asarray(arr, dtype=np.int16)
    assert a.size % 16 == 0
    if a.size == 0:
        return np.zeros((128, 1), np.int16)
    w = a.reshape(-1, 16).T.copy()
    return np.tile(w, (8, 1))


def _wrap128(arr):
    a = np.asarray(arr, dtype=np.float32)
    assert a.size % 128 == 0
    if a.size == 0:
        return np.zeros((128, 1), np.float32)
    return a.reshape(-1, 128).T.copy()


def _phi(v, n):
    """Table-row permutation: within each phase-1 store chunk of CB blocks,
    node j*128+p is stored at row p*bn+j (contiguous per-partition stores)."""
    v = np.asarray(v, np.int64)
    CB = 8
    nfull = n // P
    nch = nfull // CB
    full_end = nch * CB * P            # 49152
    part_bn = nfull - nch * CB         # blocks in the partial chunk
    part_end = nfull * P               # 49920
    out = np.empty_like(v)
    m0 = v < full_end
    q = v[m0] % (CB * P)
    out[m0] = (v[m0] // (CB * P)) * (CB * P) + (q % P) * CB + q // P
    m1 = (v >= full_end) & (v < part_end)
    q = v[m1] - full_end
    out[m1] = full_end + (q % P) * part_bn + q // P
    m2 = v >= part_end
    out[m2] = v[m2]
    return out


def host_prep(x, edge_index, W_gat, att_src, att_dst, bias_gat,
              bn1_gamma, bn1_beta, W_lin, b_lin, bn2_gamma, bn2_beta,
              n=N, e=E, num_cores=NUM_CORES):
    """Build per-core padded edge structures + constant tiles."""
    nd = n // num_cores
    nb = (nd + P - 1) // P                     # dst blocks per core
    src = np.asarray(edge_index[0], dtype=np.int64)
    dst = np.asarray(edge_index[1], dtype=np.int64)

    per_core = []
    lo_cnt = np.zeros((num_cores, nb), np.int64)
    hi_cnt = np.zeros((num_cores, nb), np.int64)
    for c in range(num_cores):
        perm = np.concatenate([
            np.arange(c * nd, (c + 1) * nd),
            np.arange(0, c * nd),
            np.arange((c + 1) * nd, n),
        ])
        pinv = np.empty(n, np.int64)
        pinv[perm] = np.arange(n)
        m = (dst >= c * nd) & (dst < (c + 1) * nd)
        es, ed = _phi(pinv[src[m]], n), dst[m] - c * nd
        blk = ed >> 7
        ishi = (es >= LO).astype(np.int64)
        order = np.lexsort((es, ishi, blk))
        es, ed, blk, ishi = es[order], ed[order], blk[order], ishi[order]
        for b in range(nb):
            bm = blk == b
            lo_cnt[c, b] = int(np.sum(bm & (ishi == 0)))
            hi_cnt[c, b] = int(np.sum(bm & (ishi == 1)))
        per_core.append((perm, es, ed, blk, ishi))

    def _pad_to(v):
        return int(-(-v // P) * P)

    m_lo = [_pad_to(int(lo_cnt[:, b].max())) for b in range(nb)]
    m_hi = [_pad_to(int(hi_cnt[:, b].max())) for b in range(nb)]
    g_b = [(m_lo[b] + m_hi[b]) // P for b in range(nb)]

    core_data = []
    for c in range(num_cores):
        perm, es, ed, blk, ishi = per_core[c]
        idx_lo, idx_hi, idx_ad, dstl = [], [], [], []
        for b in range(nb):
            bm_lo = (blk == b) & (ishi == 0)
            bm_hi = (blk == b) & (ishi == 1)
            pl = es[bm_lo]
            ph = es[bm_hi] - LO
            dl = ed[bm_lo] & 127
            dh = ed[bm_hi] & 127
            al = _phi(ed[bm_lo], n)
            ah = _phi(ed[bm_hi], n)
            npl = m_lo[b] - len(pl)
            nph = m_hi[b] - len(ph)
            idx_lo.append(np.concatenate([pl, np.zeros(npl, np.int64)]))
            idx_hi.append(np.concatenate([ph, np.zeros(nph, np.int64)]))
            idx_ad.append(np.concatenate([al, np.zeros(npl, np.int64),
                                          ah, np.zeros(nph, np.int64)]))
            dstl.append(np.concatenate([dl, np.full(npl, 300.0),
                                        dh, np.full(nph, 300.0)]))
        core_data.append(dict(
            x_t=np.ascontiguousarray(
                np.asarray(x, np.float32)[perm].T),
            idx_lo=_wrap16(np.concatenate(idx_lo)),
            idx_hi=_wrap16(np.concatenate(idx_hi)),
            dstl=_wrap128(np.concatenate(dstl)),
        ))

    # constants (shared by all cores), o-major head-interleaved layout
    import ml_dtypes
    bf = ml_dtypes.bfloat16
    for cd in core_data:
        cd["x_t"] = np.ascontiguousarray(cd["x_t"].astype(bf))

    W_gat = np.asarray(W_gat, np.float32)          # [128, 4, 64]
    att_src = np.asarray(att_src, np.float32)
    att_dst = np.asarray(att_dst, np.float32)
    V_s = np.einsum("iho,ho->ih", W_gat, att_src).astype(np.float32)
    V_d = np.einsum("iho,ho->ih", W_gat, att_dst).astype(np.float32)
    W_om = W_gat.transpose(0, 2, 1).reshape(IN_FEATS, HID)   # col o*4+h
    wvv = np.concatenate([W_om, V_s, V_d], axis=1)           # [128, 264]

    pm_idx = (np.arange(HID).reshape(HEADS, OUT_FEATS).T.reshape(-1))
    # pm_idx[o*4+h] = h*64+o : maps o-major col -> natural col
    bias_om = np.asarray(bias_gat, np.float32)[pm_idx]
    g1_om = np.asarray(bn1_gamma, np.float32)[pm_idx]
    b1_om = np.asarray(bn1_beta, np.float32)[pm_idx]
    Wl_om = np.asarray(W_lin, np.float32)[pm_idx, :]         # rows permuted

    consts = dict(
        wvv=np.ascontiguousarray(wvv).astype(bf),
        iota=np.tile(np.arange(P, dtype=np.float32)[None, :], (P, 1)),
        ident=np.eye(P, dtype=np.float32),
        ident_bf=np.eye(P, dtype=np.float32).astype(bf),
        ones_col=np.ones((P, 1), np.float32),
        zeros32=np.zeros((P, 32), np.float32),
        ones_row=np.ones((1, P), np.float32),
        g1=g1_om.reshape(2, P).T.copy(),
        b1=b1_om.reshape(2, P).T.copy(),
        g2=np.asarray(bn2_gamma, np.float32)[:, None].copy(),
        b2=np.asarray(bn2_beta, np.float32)[:, None].copy(),
        wlin=np.ascontiguousarray(
            Wl_om.reshape(2, P, OUT_FEATS).transpose(1, 0, 2)
            .reshape(P, 2 * OUT_FEATS)),
    )
    struct = dict(n=n, nd=nd, nb=nb, m_lo=m_lo, m_hi=m_hi, g_b=g_b,
                  num_cores=num_cores)
    return struct, core_data, consts


class StopPhases(Exception):
    pass


def build_kernel(struct, reps=1, skip_cc=False, stop_after=4, probe=None):
    n = struct["n"]
    nd = struct["nd"]
    nb = struct["nb"]
    m_lo = struct["m_lo"]
    m_hi = struct["m_hi"]
    g_b = struct["g_b"]
    num_cores = struct["num_cores"]
    L_lo = sum(m_lo)
    L_hi = sum(m_hi)
    L_ad = L_lo + L_hi
    G = sum(g_b)

    nc = bacc.Bacc("TRN2", debug=False, num_devices=num_cores,
                   dynamic_dma_scratch_size=49152, num_swdge_queues=3)

    # I/O
    x_t = nc.dram_tensor("x_t", [IN_FEATS, n], BF16, kind="ExternalInput")
    idx_lo = nc.dram_tensor("idx_lo", [P, max(L_lo // 16, 1)], I16, kind="ExternalInput")
    idx_hi = nc.dram_tensor("idx_hi", [P, max(L_hi // 16, 1)], I16, kind="ExternalInput")
    dstl_d = nc.dram_tensor("dstl", [P, G], F32, kind="ExternalInput")
    wvv_d = nc.dram_tensor("wvv", [IN_FEATS, HID + 8], BF16, kind="ExternalInput")
    iota_d = nc.dram_tensor("iota", [P, P], F32, kind="ExternalInput")
    ident_d = nc.dram_tensor("ident", [P, P], F32, kind="ExternalInput")
    identb_d = nc.dram_tensor("ident_bf", [P, P], BF16, kind="ExternalInput")
    onesc_d = nc.dram_tensor("ones_col", [P, 1], F32, kind="ExternalInput")
    zeros_d = nc.dram_tensor("zeros32", [P, 32], F32, kind="ExternalInput")
    onesr_d = nc.dram_tensor("ones_row", [1, P], F32, kind="ExternalInput")
    g1_d = nc.dram_tensor("g1", [P, 2], F32, kind="ExternalInput")
    b1_d = nc.dram_tensor("b1", [P, 2], F32, kind="ExternalInput")
    g2_d = nc.dram_tensor("g2", [OUT_FEATS, 1], F32, kind="ExternalInput")
    b2_d = nc.dram_tensor("b2", [OUT_FEATS, 1], F32, kind="ExternalInput")
    wlin_d = nc.dram_tensor("wlin", [P, 2 * OUT_FEATS], F32, kind="ExternalInput")
    y_d = nc.dram_tensor("y", [nd, OUT_FEATS], F32, kind="ExternalOutput")
    debug = struct.get("debug", False)
    if debug:
        dbg_h = nc.dram_tensor("dbg_h", [nd, HID], F32, kind="ExternalOutput")
        dbg_o = nc.dram_tensor("dbg_o", [nd, OUT_FEATS], F32, kind="ExternalOutput")
        dbg_st = nc.dram_tensor("dbg_st", [P, 8], F32, kind="ExternalOutput")

    # internals
    xh_ext = nc.dram_tensor("xh_ext", [n, ROW], BF16)
    asd = nc.dram_tensor("asd", [n, 64], F32)
    bn1_in = nc.dram_tensor("bn1_in", [P, 4], F32)
    bn1_out = nc.dram_tensor("bn1_out", [P, 4], F32)
    bn2_in = nc.dram_tensor("bn2_in", [OUT_FEATS, 2], F32)
    bn2_out = nc.dram_tensor("bn2_out", [OUT_FEATS, 2], F32)

    rg = [list(range(num_cores))]

    with tile.TileContext(nc) as tc:
        with tc.tile_pool(name="const", bufs=1) as cpool, \
             tc.tile_pool(name="resid", bufs=1) as rpool:
            # constants
            wvv_t = cpool.tile([IN_FEATS, HID + 8], BF16)
            nc.sync.dma_start(out=wvv_t[:], in_=wvv_d[:])
            iota_t = cpool.tile([P, P], F32)
            nc.sync.dma_start(out=iota_t[:], in_=iota_d[:])
            ident_t = cpool.tile([P, P], F32)
            nc.sync.dma_start(out=ident_t[:], in_=ident_d[:])
            identb_t = cpool.tile([P, P], BF16)
            nc.sync.dma_start(out=identb_t[:], in_=identb_d[:])
            onesc_t = cpool.tile([P, 1], F32)
            nc.sync.dma_start(out=onesc_t[:], in_=onesc_d[:])
            zeros_t = cpool.tile([P, 32], F32)
            nc.sync.dma_start(out=zeros_t[:], in_=zeros_d[:])
            onesr_t = cpool.tile([1, P], F32)
            nc.sync.dma_start(out=onesr_t[:], in_=onesr_d[:])
            g1_t = cpool.tile([P, 2], F32)
            nc.sync.dma_start(out=g1_t[:], in_=g1_d[:])
            b1_t = cpool.tile([P, 2], F32)
            nc.sync.dma_start(out=b1_t[:], in_=b1_d[:])
            g2_t = cpool.tile([OUT_FEATS, 1], F32)
            nc.sync.dma_start(out=g2_t[:], in_=g2_d[:])
            b2_t = cpool.tile([OUT_FEATS, 1], F32)
            nc.sync.dma_start(out=b2_t[:], in_=b2_d[:])
            wlin_t = cpool.tile([P, 2 * OUT_FEATS], F32)
            nc.sync.dma_start(out=wlin_t[:], in_=wlin_d[:])

            # residents
            h_res = rpool.tile([P, nb * HID], F32)
            o2_res = rpool.tile([P, nb * OUT_FEATS], F32)
            idx_lo_t = rpool.tile([P, max(L_lo // 16, 1)], I16)
            nc.sync.dma_start(out=idx_lo_t[:], in_=idx_lo[:])
            idx_hi_t = rpool.tile([P, max(L_hi // 16, 1)], I16)
            nc.sync.dma_start(out=idx_hi_t[:], in_=idx_hi[:])
            dstl_t = rpool.tile([P, G], F32)
            nc.sync.dma_start(out=dstl_t[:], in_=dstl_d[:])

            loop_cm = tc.For_i(0, reps, 1) if reps > 1 else nullcontext()
            with loop_cm:
                try:
                    # ---- phase 1: xh_ext = x_perm @ W (bf16), replicated ----
                    # DMA-batched: CH blocks per load/store to unload the SP
                    # sequencer (565ns per dma_start issue).
                    nblk1 = (n + P - 1) // P
                    CH = 8
                    nfull = n // P              # full 128-row blocks
                    nch = nfull // CH           # whole chunks of CH blocks
                    with tc.tile_pool(name="p1s", bufs=3) as p1s, \
                         tc.tile_pool(name="p1pt", bufs=2, space="PSUM") as p1pt, \
                         tc.tile_pool(name="p1pm", bufs=2, space="PSUM") as p1pm:
                        for ci in range(nch + 1):
                            b0 = ci * CH
                            bn_ = min(CH, nfull - b0)
                            if bn_ <= 0:
                                break
                            r0 = b0 * P
                            xb = p1s.tile([P, CH * P], BF16, tag="xb")
                            nc.sync.dma_start(
                                out=xb[:, 0:bn_ * P],
                                in_=x_t[:, r0:r0 + bn_ * P])
                            sbx = p1s.tile([P, CH, ROW], BF16, tag="sbx")
                            sba = p1s.tile([P, CH, 64], F32, tag="sba")
                            for j in range(bn_):
                                i = b0 + j
                                pm = p1pm.tile([P, HID + 8], F32, tag="pm")
                                nc.tensor.matmul(out=pm[:],
                                                 lhsT=xb[:, j * P:(j + 1) * P],
                                                 rhs=wvv_t[:],
                                                 start=True, stop=True)
                                if i % 2 == 0:
                                    nc.scalar.copy(sbx[:, j, 0:HID + 4],
                                                   pm[:, 0:HID + 4])
                                    nc.vector.tensor_copy(sba[:, j, 0:8],
                                                          pm[:, HID:HID + 8])
                                else:
                                    nc.vector.tensor_copy(sbx[:, j, 0:HID + 4],
                                                          pm[:, 0:HID + 4])
                                    nc.scalar.copy(sba[:, j, 0:8],
                                                   pm[:, HID:HID + 8])
                            if probe != "nostore":
                                # phi layout: DRAM row r0+p*bn_+j <- node r0+j*128+p;
                                # full rows so each partition's bn_ rows coalesce
                                nc.sync.dma_start(
                                    out=xh_ext[r0:r0 + bn_ * P, :].rearrange(
                                        "(p j) f -> p j f", j=bn_),
                                    in_=sbx[:, 0:bn_, :])
                                nc.sync.dma_start(
                                    out=asd[r0:r0 + bn_ * P, :].rearrange(
                                        "(p j) f -> p j f", j=bn_),
                                    in_=sba[:, 0:bn_, :])
                            else:
                                nc.sync.dma_start(out=xh_ext[r0:r0 + P, 0:HID + 4],
                                                  in_=sbx[:, 0, 0:HID + 4])
                        if n % P:
                            r0 = nfull * P
                            rn = n - r0
                            xb = p1s.tile([P, CH * P], BF16, tag="xb")
                            nc.sync.dma_start(out=xb[:, 0:rn],
                                              in_=x_t[:, r0:r0 + rn])
                            pm = p1pm.tile([P, HID + 8], F32, tag="pm")
                            nc.tensor.matmul(out=pm[:rn], lhsT=xb[:, 0:rn], rhs=wvv_t[:],
                                             start=True, stop=True)
                            sbx = p1s.tile([P, CH, ROW], BF16, tag="sbx")
                            sba = p1s.tile([P, CH, 64], F32, tag="sba")
                            nc.scalar.copy(sbx[:rn, 0, 0:HID + 4], pm[:rn, 0:HID + 4])
                            nc.vector.tensor_copy(sba[:rn, 0, 0:8],
                                                  pm[:rn, HID:HID + 8])
                            nc.sync.dma_start(out=xh_ext[r0:r0 + rn, 0:HID + 4],
                                              in_=sbx[:rn, 0, 0:HID + 4])
                            nc.sync.dma_start(out=asd[r0:r0 + rn, 0:8],
                                              in_=sba[:rn, 0, 0:8])

                    # ---- phase 2: edge aggregation ----
                    if stop_after < 2:
                        raise StopPhases
                    with tc.tile_pool(name="p2g", bufs=2) as p2g, \
                         tc.tile_pool(name="p2i", bufs=4) as p2i, \
                         tc.tile_pool(name="p2s", bufs=3) as p2s, \
                         tc.tile_pool(name="p2p", bufs=2, space="PSUM") as p2p, \
                         tc.tile_pool(name="p2t", bufs=2, space="PSUM") as p2t, \
                         tc.tile_pool(name="p2ad", bufs=2, space="PSUM") as p2ad, \
                         tc.tile_pool(name="p2st", bufs=1, space="PSUM") as p2st:
                        stats_ps = p2st.tile([P, 4], F32, tag="st", name="st")
                        nc.vector.tensor_copy(stats_ps[:], zeros_t[:, 0:4])
                        off_lo = 0
                        off_hi = 0
                        off_ad = 0
                        gof = 0
                        ISUB = 7  # indicator groups per DVE op
                        gmax = max(g_b)
                        for b in range(nb):
                            nd_b = min(P, nd - b * P)
                            glo = m_lo[b] // P
                            ghi = m_hi[b] // P
                            gb = g_b[b]
                            gath = p2g.tile([P, gmax, ROW], BF16, tag="gath")
                            if probe in ("xh512", "none"):
                                gp = p2g.tile([P, gmax, 256], BF16, tag="gprobe")
                                if m_lo[b] > 0:
                                    nc.gpsimd.dma_gather(
                                        out_ap=gp[:, 0:glo, :], in_ap=xh_ext[0:min(LO, n), 0:256],
                                        idxs_ap=idx_lo_t[:, off_lo:off_lo + m_lo[b] // 16],
                                        num_idxs=m_lo[b], num_idxs_reg=m_lo[b],
                                        elem_size=256, elem_step=ROW, single_packet=False)
                                if m_hi[b] > 0:
                                    nc.gpsimd.dma_gather(
                                        out_ap=gp[:, glo:gb, :], in_ap=xh_ext[LO:n, 0:256],
                                        idxs_ap=idx_hi_t[:, off_hi:off_hi + m_hi[b] // 16],
                                        num_idxs=m_hi[b], num_idxs_reg=m_hi[b],
                                        elem_size=256, elem_step=ROW, single_packet=False)
                            else:
                                if m_lo[b] > 0:
                                    nc.gpsimd.dma_gather(
                                        out_ap=gath[:, 0:glo, :], in_ap=xh_ext[0:min(LO, n), :],
                                        idxs_ap=idx_lo_t[:, off_lo:off_lo + m_lo[b] // 16],
                                        num_idxs=m_lo[b], num_idxs_reg=m_lo[b],
                                        elem_size=ROW, single_packet=False,
                                        queue_num=0)
                                if m_hi[b] > 0:
                                    nc.gpsimd.dma_gather(
                                        out_ap=gath[:, glo:gb, :], in_ap=xh_ext[LO:n, :],
                                        idxs_ap=idx_hi_t[:, off_hi:off_hi + m_hi[b] // 16],
                                        num_idxs=m_hi[b], num_idxs_reg=m_hi[b],
                                        elem_size=ROW, single_packet=False,
                                        queue_num=1)
                            # self-loop data (phi layout: one load per 8 blocks)
                            if b % 8 == 0:
                                c0 = (b // 8) * 8 * P
                                xh_blk8 = p2s.tile([P, 8, ROW], BF16, tag="xhb8")
                                nc.sync.dma_start(
                                    out=xh_blk8[:],
                                    in_=xh_ext[c0:c0 + 8 * P, :].rearrange(
                                        "(p j) f -> p j f", j=8))
                                asd_blk8 = p2s.tile([P, 8, 64], F32, tag="asdb8")
                                nc.sync.dma_start(
                                    out=asd_blk8[:],
                                    in_=asd[c0:c0 + 8 * P, :].rearrange(
                                        "(p j) f -> p j f", j=8))
                            xh_blk = xh_blk8[:, b % 8, :]
                            asd_blk = asd_blk8[:, b % 8, 0:8]

                            if probe in ("p2a", "p2a_noad"):
                                off_lo += m_lo[b] // 16
                                off_hi += m_hi[b] // 16
                                gof += gb
                                continue
                            # indicators (ISUB groups per build op) + per-group
                            # a_d via transposed-indicator matmul into adps;
                            # scale per batch so PE matmuls overlap DVE scaling
                            ee = p2s.tile([P, gmax, 4], BF16, tag="ee")
                            ad4 = p2s.tile([P, 4], BF16, tag="ad4")
                            nc.vector.tensor_copy(ad4[:nd_b], asd_blk[:nd_b, 4:8])
                            ind = []
                            for j0 in range(0, gb, ISUB):
                                j1 = min(j0 + ISUB, gb)
                                it = p2i.tile([P, ISUB, P], BF16, tag="ind")
                                nc.vector.tensor_tensor(
                                    it[:, 0:j1 - j0, :],
                                    iota_t[:, None, :].to_broadcast([P, j1 - j0, P]),
                                    dstl_t[:, gof + j0:gof + j1, None].to_broadcast(
                                        [P, j1 - j0, P]),
                                    OP.is_equal)
                                ind.append((j0, it))
                                adps = p2ad.tile([P, 4 * ISUB], F32, tag="adps")
                                nc.vector.tensor_copy(adps[:, 0:4 * (j1 - j0)],
                                                      zeros_t[:, 0:4 * (j1 - j0)])
                                for g in range(j0, j1):
                                    ptT = p2t.tile([P, P], BF16, tag="ptT")
                                    nc.tensor.transpose(out=ptT[:], in_=it[:, g - j0, :],
                                                        identity=identb_t[:])
                                    indT = p2s.tile([P, P], BF16, tag="indT")
                                    if g % 2 == 0:
                                        nc.scalar.copy(indT[:], ptT[:])
                                    else:
                                        nc.vector.tensor_copy(indT[:], ptT[:])
                                    nc.tensor.matmul(
                                        out=adps[:, 4 * (g - j0):4 * (g - j0) + 4],
                                        lhsT=indT[:], rhs=ad4[:],
                                        start=False, stop=True)
                                # ee = exp(leaky(a_s + a_d)), then scale this batch
                                nc.vector.tensor_tensor(
                                    ee[:, j0:j1, :],
                                    gath[:, j0:j1, HID:HID + 4],
                                    adps[:, 0:4 * (j1 - j0)].rearrange(
                                        "p (g q) -> p g q", q=4),
                                    OP.add)
                                nc.vector.scalar_tensor_tensor(
                                    ee[:, j0:j1, :], ee[:, j0:j1, :], NEG_SLOPE,
                                    ee[:, j0:j1, :], OP.mult, OP.max)
                                nc.scalar.activation(ee[:, j0:j1, :], ee[:, j0:j1, :],
                                                     AF.Exp)
                                nc.vector.tensor_tensor(
                                    gath[:, j0:j1, 0:HID].rearrange(
                                        "p g (o h) -> p g o h", h=HEADS),
                                    gath[:, j0:j1, 0:HID].rearrange(
                                        "p g (o h) -> p g o h", h=HEADS),
                                    ee[:, j0:j1, None, :].to_broadcast(
                                        [P, j1 - j0, OUT_FEATS, HEADS]),
                                    OP.mult)
                                nc.scalar.copy(gath[:, j0:j1, HID:HID + 4],
                                               ee[:, j0:j1, :])
                            if probe in ("p2b", "p2c"):
                                off_lo += m_lo[b] // 16
                                off_hi += m_hi[b] // 16
                                gof += gb
                                continue
                            psb = p2p.tile([P, HID + 4], F32, tag="psb")
                            for g in range(gb):
                                it = ind[g // ISUB][1]
                                nc.tensor.matmul(
                                    out=psb[:nd_b], lhsT=it[:, g % ISUB, 0:nd_b],
                                    rhs=gath[:, g, 0:HID + 4],
                                    start=(g == 0), stop=(g == gb - 1))

                            # epilogue: self loops, normalize, bias, h, stats
                            ee_s = p2s.tile([P, 4], F32, tag="ees")
                            nc.vector.tensor_tensor(ee_s[:nd_b], asd_blk[:nd_b, 0:4],
                                                    asd_blk[:nd_b, 4:8], OP.add)
                            nc.vector.scalar_tensor_tensor(
                                ee_s[:nd_b], ee_s[:nd_b], NEG_SLOPE, ee_s[:nd_b],
                                OP.mult, OP.max)
                            nc.scalar.activation(ee_s[:nd_b], ee_s[:nd_b], AF.Exp)
                            den = p2s.tile([P, 4], F32, tag="den")
                            nc.vector.tensor_tensor(den[:nd_b], psb[:nd_b, HID:HID + 4],
                                                    ee_s[:nd_b], OP.add)
                            rec = p2s.tile([P, 4], F32, tag="rec")
                            nc.vector.reciprocal(rec[:nd_b], den[:nd_b])
                            t1 = p2s.tile([P, HID], F32, tag="t1")
                            nc.vector.tensor_tensor(
                                t1[:nd_b].rearrange("p (o h) -> p o h", h=HEADS),
                                xh_blk[:nd_b, 0:HID].rearrange("p (o h) -> p o h", h=HEADS),
                                ee_s[:nd_b, None, :].to_broadcast([nd_b, OUT_FEATS, HEADS]),
                                OP.mult)
                            nc.vector.tensor_tensor(t1[:nd_b], t1[:nd_b],
                                                    psb[:nd_b, 0:HID], OP.add)
                            hslot = h_res[:, b * HID:(b + 1) * HID]
                            nc.vector.tensor_tensor(
                                hslot[:nd_b].rearrange("p (o h) -> p o h", h=HEADS),
                                t1[:nd_b].rearrange("p (o h) -> p o h", h=HEADS),
                                rec[:nd_b, None, :].to_broadcast([nd_b, OUT_FEATS, HEADS]),
                                OP.mult)
                            if debug:
                                nc.sync.dma_start(out=dbg_h[b * P:b * P + nd_b, :],
                                                  in_=hslot[:nd_b])
                            sq = p2s.tile([P, HID], F32, tag="sq")
                            nc.scalar.activation(sq[:nd_b], hslot[:nd_b], AF.Square)
                            for k in range(2):
                                nc.tensor.matmul(out=stats_ps[:, k:k + 1],
                                                 lhsT=hslot[:nd_b, k * P:(k + 1) * P],
                                                 rhs=onesc_t[:nd_b],
                                                 start=False, stop=True)
                                nc.tensor.matmul(out=stats_ps[:, 2 + k:3 + k],
                                                 lhsT=sq[:nd_b, k * P:(k + 1) * P],
                                                 rhs=onesc_t[:nd_b],
                                                 start=False, stop=True)
                            off_lo += m_lo[b] // 16
                            off_hi += m_hi[b] // 16
                            gof += gb

                        if probe in ("p2a", "p2a_noad", "p2b", "p2c"):
                            raise StopPhases
                        # BN1 stats allreduce + s,t
                        st_sb = p2s.tile([P, 4], F32, tag="stsb")
                        nc.vector.tensor_copy(st_sb[:], stats_ps[:])
                        nc.sync.dma_start(out=bn1_in[:], in_=st_sb[:])
                        if not skip_cc:
                            nc.gpsimd.collective_compute(
                                "AllReduce", OP.add, replica_groups=rg,
                                ins=[bn1_in[:]], outs=[bn1_out[:]])
                        else:
                            nc.sync.dma_start(out=bn1_out[:], in_=st_sb[:])
                        st_g = p2s.tile([P, 4], F32, tag="stg")
                        nc.sync.dma_start(out=st_g[:], in_=bn1_out[:])

                    if stop_after < 3:
                        raise StopPhases
                    with tc.tile_pool(name="p3s", bufs=3) as p3s, \
                         tc.tile_pool(name="bc", bufs=1) as bc, \
                         tc.tile_pool(name="p3pt", bufs=2, space="PSUM") as p3pt, \
                         tc.tile_pool(name="p3po", bufs=2, space="PSUM") as p3po, \
                         tc.tile_pool(name="p3st", bufs=1, space="PSUM") as p3st, \
                         tc.tile_pool(name="p3bc", bufs=1, space="PSUM") as p3bc:
                        mean = p3s.tile([P, 2], F32, tag="mean")
                        nc.scalar.mul(mean[:], st_g[:, 0:2], 1.0 / n)
                        esq = p3s.tile([P, 2], F32, tag="esq")
                        nc.scalar.mul(esq[:], st_g[:, 2:4], 1.0 / n)
                        var = p3s.tile([P, 2], F32, tag="var")
                        nc.vector.tensor_tensor(var[:], mean[:], mean[:], OP.mult)
                        nc.vector.tensor_tensor(var[:], esq[:], var[:], OP.subtract)
                        nc.vector.tensor_scalar_add(var[:], var[:], EPS)
                        sdv = p3s.tile([P, 2], F32, tag="sdv")
                        nc.scalar.activation(sdv[:], var[:], AF.Sqrt)
                        inv = p3s.tile([P, 2], F32, tag="inv")
                        nc.vector.reciprocal(inv[:], sdv[:])
                        s1 = p3s.tile([P, 2], F32, tag="s1")
                        nc.vector.tensor_tensor(s1[:], inv[:], g1_t[:], OP.mult)
                        tsh = p3s.tile([P, 2], F32, tag="tsh")
                        nc.vector.tensor_tensor(tsh[:], mean[:], s1[:], OP.mult)
                        nc.vector.tensor_tensor(tsh[:], b1_t[:], tsh[:], OP.subtract)

                        # broadcast s1/tsh to node-major [P, 256]
                        s_bc = bc.tile([P, HID], F32)
                        t_bc = bc.tile([P, HID], F32)
                        for (vec, dstt) in ((s1, s_bc), (tsh, t_bc)):
                            for k in range(2):
                                row = p3s.tile([1, P], F32, tag="row")
                                nc.sync.dma_start(out=row[:], in_=vec[:, k:k + 1])
                                pbc = p3bc.tile([P, P], F32, tag="pbc")
                                nc.tensor.matmul(out=pbc[:], lhsT=onesr_t[:], rhs=row[:],
                                                 start=True, stop=True)
                                nc.scalar.copy(dstt[:, k * P:(k + 1) * P], pbc[:])

                        if debug:
                            nc.sync.dma_start(out=dbg_st[:, 0:4], in_=st_g[:])
                            nc.sync.dma_start(out=dbg_st[:, 4:6], in_=s1[:])
                            nc.sync.dma_start(out=dbg_st[:, 6:8], in_=tsh[:])
                        # ---- phase 3: BN1 + relu + linear + BN2 stats ----
                        ps_st2 = [p3st.tile([OUT_FEATS, 1], F32, tag=f"st2{j}",
                                            name=f"st2{j}")[:] for j in range(2)]
                        for b in range(nb):
                            nd_b = min(P, nd - b * P)
                            hslot = h_res[:, b * HID:(b + 1) * HID]
                            hb = p3s.tile([P, HID], F32, tag="hb")
                            nc.vector.tensor_tensor(hb[:nd_b], hslot[:nd_b], s_bc[:nd_b],
                                                    OP.mult)
                            nc.vector.tensor_tensor(hb[:nd_b], hb[:nd_b], t_bc[:nd_b],
                                                    OP.add)
                            nc.vector.tensor_scalar(hb[:nd_b], hb[:nd_b], 0.0, None,
                                                    OP.max)
                            po = p3po.tile([P, OUT_FEATS], F32, tag="po")
                            for k in range(2):
                                ptr = p3pt.tile([P, P], F32, tag="tr")
                                nc.tensor.transpose(out=ptr[:, :nd_b],
                                                    in_=hb[:nd_b, k * P:(k + 1) * P],
                                                    identity=ident_t[:nd_b, :nd_b])
                                hbt = p3s.tile([P, P], F32, tag="hbt")
                                if k == 0:
                                    nc.scalar.copy(hbt[:, :nd_b], ptr[:, :nd_b])
                                else:
                                    nc.vector.tensor_copy(hbt[:, :nd_b], ptr[:, :nd_b])
                                nc.tensor.matmul(out=po[:nd_b], lhsT=hbt[:, :nd_b],
                                                 rhs=wlin_t[:, k * OUT_FEATS:(k + 1) * OUT_FEATS],
                                                 start=(k == 0), stop=(k == 1))
                            oslot = o2_res[:, b * OUT_FEATS:(b + 1) * OUT_FEATS]
                            if b % 2 == 0:
                                nc.vector.tensor_copy(oslot[:nd_b], po[:nd_b])
                            else:
                                nc.scalar.copy(oslot[:nd_b], po[:nd_b])
                            if debug:
                                nc.sync.dma_start(out=dbg_o[b * P:b * P + nd_b, :],
                                                  in_=oslot[:nd_b])
                            sq2 = p3s.tile([P, OUT_FEATS], F32, tag="sq2")
                            nc.vector.tensor_tensor(sq2[:nd_b], oslot[:nd_b],
                                                    oslot[:nd_b], OP.mult)
                            nc.tensor.matmul(out=ps_st2[0], lhsT=oslot[:nd_b],
                                             rhs=onesc_t[:nd_b],
                                             start=(b == 0), stop=(b == nb - 1))
                            nc.tensor.matmul(out=ps_st2[1], lhsT=sq2[:nd_b],
                                             rhs=onesc_t[:nd_b],
                                             start=(b == 0), stop=(b == nb - 1))

                        st2_sb = p3s.tile([OUT_FEATS, 2], F32, tag="st2sb")
                        for j in range(2):
                            nc.vector.tensor_copy(st2_sb[:, j:j + 1], ps_st2[j])
                        nc.sync.dma_start(out=bn2_in[:], in_=st2_sb[:])
                        if not skip_cc:
                            nc.gpsimd.collective_compute(
                                "AllReduce", OP.add, replica_groups=rg,
                                ins=[bn2_in[:]], outs=[bn2_out[:]])
                        else:
                            nc.sync.dma_start(out=bn2_out[:], in_=st2_sb[:])
                        st2_g = p3s.tile([OUT_FEATS, 2], F32, tag="st2g")
                        nc.sync.dma_start(out=st2_g[:], in_=bn2_out[:])

                        mean2 = p3s.tile([OUT_FEATS, 1], F32, tag="mean2")
                        nc.scalar.mul(mean2[:], st2_g[:, 0:1], 1.0 / n)
                        esq2 = p3s.tile([OUT_FEATS, 1], F32, tag="esq2")
                        nc.scalar.mul(esq2[:], st2_g[:, 1:2], 1.0 / n)
                        var2 = p3s.tile([OUT_FEATS, 1], F32, tag="var2")
                        nc.vector.tensor_tensor(var2[:], mean2[:], mean2[:], OP.mult)
                        nc.vector.tensor_tensor(var2[:], esq2[:], var2[:], OP.subtract)
                        nc.vector.tensor_scalar_add(var2[:], var2[:], EPS)
                        sdv2 = p3s.tile([OUT_FEATS, 1], F32, tag="sdv2")
                        nc.scalar.activation(sdv2[:], var2[:], AF.Sqrt)
                        inv2 = p3s.tile([OUT_FEATS, 1], F32, tag="inv2")
                        nc.vector.reciprocal(inv2[:], sdv2[:])
                        s2 = p3s.tile([OUT_FEATS, 1], F32, tag="s2")
                        nc.vector.tensor_tensor(s2[:], inv2[:], g2_t[:], OP.mult)
                        t2 = p3s.tile([OUT_FEATS, 1], F32, tag="t2")
                        nc.vector.tensor_tensor(t2[:], mean2[:], s2[:], OP.mult)
                        nc.vector.tensor_tensor(t2[:], b2_t[:], t2[:], OP.subtract)

                        s2_bc = bc.tile([P, OUT_FEATS], F32)
                        t2_bc = bc.tile([P, OUT_FEATS], F32)
                        for (vec, dstt) in ((s2, s2_bc), (t2, t2_bc)):
                            row = p3s.tile([1, OUT_FEATS], F32, tag="row2")
                            nc.sync.dma_start(out=row[:], in_=vec[:])
                            pbc = p3bc.tile([P, P], F32, tag="pbc")
                            nc.tensor.matmul(out=pbc[:, 0:OUT_FEATS], lhsT=onesr_t[:],
                                             rhs=row[:], start=True, stop=True)
                            nc.scalar.copy(dstt[:], pbc[:, 0:OUT_FEATS])

                        # ---- phase 4: BN2 apply + relu + store ----
                        for b in range(nb):
                            nd_b = min(P, nd - b * P)
                            oslot = o2_res[:, b * OUT_FEATS:(b + 1) * OUT_FEATS]
                            ob = p3s.tile([P, OUT_FEATS], F32, tag="ob")
                            nc.vector.tensor_tensor(ob[:nd_b], oslot[:nd_b], s2_bc[:nd_b],
                                                    OP.mult)
                            nc.vector.tensor_tensor(ob[:nd_b], ob[:nd_b], t2_bc[:nd_b],
                                                    OP.add)
                            nc.vector.tensor_scalar(ob[:nd_b], ob[:nd_b], 0.0, None,
                                                    OP.max)
                            nc.sync.dma_start(out=y_d[b * P:b * P + nd_b, :],
                                              in_=ob[:nd_b])

                except StopPhases:
                    pass
    nc.compile()
    return nc


def _legalize_waits(nc, max_waits=1):
    """This walrus build encodes at most one sync-wait per instruction; move
    extra waits onto preceding NoOps on the same engine."""
    nsplit = 0
    for bb in nc.main_func.blocks:
        new = []
        for ins in bb.instructions:
            si = ins.sync_info
            if si is not None and len(si.on_wait) > max_waits:
                waits = list(si.on_wait)
                for j, w in enumerate(waits[max_waits:]):
                    nop = mybir.InstNoOp(
                        name=f"{ins.name}_wsplit{j}", ins=[], outs=[],
                        engine=ins.engine,
                        sync_info=mybir.SyncInfo(on_wait=[w], on_update=[]),
                    )
                    new.append(nop)
                    nsplit += 1
                si.on_wait = waits[:max_waits]
            new.append(ins)
        bb.instructions[:] = new
    return nsplit


def kernel(**inputs):
    x = np.asarray(inputs["x"], np.float32)
    edge_index = np.asarray(inputs["edge_index"])
    struct, core_data, consts = host_prep(
        x, edge_index, inputs["W_gat"], inputs["att_src"], inputs["att_dst"],
        inputs["bias_gat"], inputs["bn1_gamma"], inputs["bn1_beta"],
        inputs["W_lin"], inputs["b_lin"], inputs["bn2_gamma"], inputs["bn2_beta"])
    nc = build_kernel(struct)
    _legalize_waits(nc)
    in_maps = []
    for c in range(struct["num_cores"]):
        m = dict(consts)
        m.update(core_data[c])
        in_maps.append(m)
    res = run_bass_kernel_spmd(nc, in_maps, list(range(struct["num_cores"])))
    out = np.concatenate([res.results[c]["y"] for c in range(struct["num_cores"])],
                         axis=0)
    return out.astype(np.float32)
